# revision 1
# baseline (speedup 1.0000x reference)
"""GAT DirSeq (conv_in + conv_out on flipped edges) Trainium2 kernel.

Strategy (edge partition by destination block, per sharding hint):
  - Nodes are grouped into 128-node blocks; blocks are sharded over 8 cores.
  - Node phase (replicated on every core): T_in = [h_in | a_src_in],
    T_out = [h_out | a_src_out] (fp16, DRAM tables), A = [a_dst_in | a_dst_out].
    h = x @ W computed by TensorE; the per-head alpha reductions are folded
    into the same matmul as extra columns (V = W_head @ a_head).
  - Edge phase: per destination block, indirect-DMA gather of source rows,
    softmax expressed as unnormalized weighted sum (exact same math as the
    reference since the max-subtraction cancels): num = sum ex*h, den = sum ex,
    scattered into PSUM via one-hot matmuls; out = num/den + bias.
"""

import math
from contextlib import ExitStack

import numpy as np

N = 100000
E = 800000
_LAST = {}
D_IN = 128
HEADS = 8
C = 16
NEG_SLOPE = 0.2
P = 128


# ---------------------------------------------------------------- host prep
def _edge_arrays(key, oth, NB, SENT):
    """Per-direction edge layout: for each 128-node block of `key`, the list of
    edges targeting it, padded to S*128, laid out so edge at (partition p,
    slot s) is sorted position s*128+p within the block."""
    order = np.argsort(key, kind="stable")
    k_s = key[order].astype(np.int64)
    o_s = oth[order].astype(np.int64)
    blk = k_s // P
    cnt = np.bincount(blk, minlength=NB)
    S = max(1, int(math.ceil(cnt.max() / P)))
    starts = np.zeros(NB + 1, np.int64)
    np.cumsum(cnt, out=starts[1:])
    pos = np.arange(k_s.size, dtype=np.int64) - starts[blk]
    p = pos % P
    s = pos // P
    gidx = np.full((NB, P, S), SENT, np.int32)
    gdst = np.full((NB, P, S), SENT, np.int32)
    ldst = np.zeros((NB, P, S), np.float16)
    gidx[blk, p, s] = o_s
    gdst[blk, p, s] = k_s
    ldst[blk, p, s] = (k_s % P).astype(np.float16)
    return gidx, gdst, ldst, S


def _build_and_run(x, ei, W_in, a_src_in, a_dst_in, b_in, W_out, a_src_out,
                   a_dst_out, b_out, n_nodes, n_edges, n_cores=8):
    import concourse.bacc as bacc
    import concourse.bass as bass
    import concourse.mybir as mybir
    import concourse.tile as tile
    from concourse.bass_utils import run_bass_kernel_spmd

    fp16 = mybir.dt.float16
    f32 = mybir.dt.float32
    i32 = mybir.dt.int32

    NBLK_C = int(math.ceil(n_nodes / P / n_cores))  # blocks per core
    NB = NBLK_C * n_cores
    NPAD = NB * P
    SENT = NPAD  # sentinel row id
    NR = NPAD + 1

    src, dst = ei[0].astype(np.int64), ei[1].astype(np.int64)
    gi_i, gd_i, ld_i, S_IN = _edge_arrays(dst, src, NB, SENT)
    gi_o, gd_o, ld_o, S_OUT = _edge_arrays(src, dst, NB, SENT)
    SMAX = max(S_IN, S_OUT)
    KI = 2 * (S_IN + S_OUT)
    KL = S_IN + S_OUT

    # folded parameter matrix [D_IN, 288]
    Vsrc_in = np.stack([W_in[:, h * C:(h + 1) * C] @ a_src_in[h] for h in range(HEADS)], 1)
    Vdst_in = np.stack([W_in[:, h * C:(h + 1) * C] @ a_dst_in[h] for h in range(HEADS)], 1)
    Vsrc_out = np.stack([W_out[:, h * C:(h + 1) * C] @ a_src_out[h] for h in range(HEADS)], 1)
    Vdst_out = np.stack([W_out[:, h * C:(h + 1) * C] @ a_dst_out[h] for h in range(HEADS)], 1)
    wcat = np.concatenate(
        [W_in, Vsrc_in, W_out, Vsrc_out, Vdst_in, Vdst_out], axis=1
    ).astype(np.float16)  # [128, 288]

    xT = np.zeros((D_IN, NPAD), np.float16)
    xT[:, :n_nodes] = x.T.astype(np.float16)

    iota = np.tile(np.arange(P, dtype=np.float16), (P, SMAX)).reshape(P, SMAX * P)
    sent = np.zeros((1, 288), np.float16)
    sent[0, 128:136] = -1e4   # T_in alpha_src
    sent[0, 264:272] = -1e4   # T_out alpha_src
    bias = np.tile((b_in + b_out).astype(np.float32)[None, :], (P, 1))

    # per-core edge info, SBUF layout [128, NBLK_C * K]
    einfo, ldsta = [], []
    for k in range(n_cores):
        sl = slice(k * NBLK_C, (k + 1) * NBLK_C)
        e = np.concatenate([gi_i[sl], gd_i[sl], gi_o[sl], gd_o[sl]], axis=2)
        einfo.append(np.ascontiguousarray(e.transpose(1, 0, 2).reshape(P, NBLK_C * KI)))
        l = np.concatenate([ld_i[sl], ld_o[sl]], axis=2)
        ldsta.append(np.ascontiguousarray(l.transpose(1, 0, 2).reshape(P, NBLK_C * KL)))

    # ------------------------------------------------------------- program
    nc = bacc.Bacc(None, target_bir_lowering=False, debug=False)
    ctx = ExitStack()

    p_xT = nc.declare_dram_parameter("xT", [P, NPAD], fp16, isOutput=False)
    p_wcat = nc.declare_dram_parameter("wcat", [P, 288], fp16, isOutput=False)
    p_iota = nc.declare_dram_parameter("iota", [P, SMAX * P], fp16, isOutput=False)
    p_sent = nc.declare_dram_parameter("sent", [1, 288], fp16, isOutput=False)
    p_bias = nc.declare_dram_parameter("bias", [P, 128], f32, isOutput=False)
    p_einfo = nc.declare_dram_parameter("einfo", [P, NBLK_C * KI], i32, isOutput=False)
    p_ldst = nc.declare_dram_parameter("ldst", [P, NBLK_C * KL], fp16, isOutput=False)
    p_out = nc.declare_dram_parameter("out", [NBLK_C * P, 128], f32, isOutput=True)

    T_in = nc.dram_tensor("T_in", [NR, 136], fp16)
    T_out = nc.dram_tensor("T_out", [NR, 136], fp16)
    A = nc.dram_tensor("A", [NR, 16], fp16)

    NBAT = 16

    with tile.TileContext(nc) as tc:
        with (
            tc.tile_pool(name="const", bufs=1) as cpool,
            tc.tile_pool(name="xc", bufs=2) as xpool,
            tc.tile_pool(name="nstage", bufs=2) as spool,
            tc.tile_pool(name="npsum", bufs=2, space="PSUM") as npsum,
            tc.tile_pool(name="gath", bufs=3) as gpool,
            tc.tile_pool(name="agath", bufs=3) as apool,
            tc.tile_pool(name="oneh", bufs=3) as opool,
            tc.tile_pool(name="small", bufs=4) as mpool,
            tc.tile_pool(name="epsum", bufs=2, space="PSUM") as epsum,
            tc.tile_pool(name="epi", bufs=3) as dpool,
        ):
            wcat_s = cpool.tile([P, 288], fp16)
            nc.sync.dma_start(out=wcat_s[:], in_=p_wcat[:])
            iota_s = cpool.tile([P, SMAX * P], fp16)
            nc.sync.dma_start(out=iota_s[:], in_=p_iota[:])
            bias_s = cpool.tile([P, 128], f32)
            nc.sync.dma_start(out=bias_s[:], in_=p_bias[:])
            einfo_s = cpool.tile([P, NBLK_C * KI], i32)
            nc.sync.dma_start(out=einfo_s[:], in_=p_einfo[:])
            ldst_s = cpool.tile([P, NBLK_C * KL], fp16)
            nc.sync.dma_start(out=ldst_s[:], in_=p_ldst[:])

            # sentinel rows
            nc.sync.dma_start(out=T_in[NPAD:NR, :], in_=p_sent[:, 0:136])
            nc.sync.dma_start(out=T_out[NPAD:NR, :], in_=p_sent[:, 136:272])
            nc.sync.dma_start(out=A[NPAD:NR, :], in_=p_sent[:, 272:288])

            # ---------------- node phase ----------------
            for g0 in range(0, NB, NBAT):
                nb = min(NBAT, NB - g0)
                xc = xpool.tile([P, nb * P], fp16, tag="xc")
                nc.sync.dma_start(out=xc[:], in_=p_xT[:, g0 * P:(g0 + nb) * P])
                stage = spool.tile([P, nb * 288], fp16, tag="nstage")
                for j in range(nb):
                    ps = npsum.tile([P, 288], f32, tag="nps")
                    nc.tensor.matmul(out=ps[:], lhsT=xc[:, j * P:(j + 1) * P],
                                     rhs=wcat_s[:], start=True, stop=True)
                    dstg = stage[:, j * 288:(j + 1) * 288]
                    if j % 2 == 0:
                        nc.scalar.copy(out=dstg, in_=ps[:])
                    else:
                        nc.vector.tensor_copy(out=dstg, in_=ps[:])
                st3 = stage[:].rearrange("p (j c) -> p j c", c=288)
                r0 = g0 * P
                rows = nb * P
                tin_v = T_in[r0:r0 + rows, :].rearrange("(j p) c -> p j c", p=P)
                nc.sync.dma_start(out=tin_v, in_=st3[:, :, 0:136])
                tout_v = T_out[r0:r0 + rows, :].rearrange("(j p) c -> p j c", p=P)
                nc.sync.dma_start(out=tout_v, in_=st3[:, :, 136:272])
                a_v = A[r0:r0 + rows, :].rearrange("(j p) c -> p j c", p=P)
                nc.sync.dma_start(out=a_v, in_=st3[:, :, 272:288])

            # ---------------- edge phase ----------------
            for b in range(NBLK_C):
                outs_d = []
                for d in range(2):
                    S = S_IN if d == 0 else S_OUT
                    T = T_in if d == 0 else T_out
                    eoff = b * KI + (0 if d == 0 else 2 * S_IN)
                    loff = b * KL + (0 if d == 0 else S_IN)

                    gath = gpool.tile([P, S * 136], fp16, tag="gath")
                    for s in range(S):
                        nc.gpsimd.indirect_dma_start(
                            out=gath[:, s * 136:(s + 1) * 136], out_offset=None,
                            in_=T[:, :],
                            in_offset=bass.IndirectOffsetOnAxis(
                                ap=einfo_s[:, eoff + s:eoff + s + 1], axis=0))
                    agath = apool.tile([P, S * 16], fp16, tag="agath")
                    for s in range(S):
                        nc.gpsimd.indirect_dma_start(
                            out=agath[:, s * 16:(s + 1) * 16], out_offset=None,
                            in_=A[:, :],
                            in_offset=bass.IndirectOffsetOnAxis(
                                ap=einfo_s[:, eoff + S + s:eoff + S + s + 1],
                                axis=0))

                    oneh = opool.tile([P, S * P], fp16, tag="oneh")
                    ld_b = ldst_s[:, loff:loff + S].rearrange("p (s o) -> p s o", o=1)
                    nc.vector.tensor_tensor(
                        out=oneh[:].rearrange("p (s e) -> p s e", e=P),
                        in0=ld_b.to_broadcast([P, S, P]),
                        in1=iota_s[:, 0:S * P].rearrange("p (s e) -> p s e", e=P),
                        op=mybir.AluOpType.is_equal)

                    g3 = gath[:].rearrange("p (s c) -> p s c", c=136)
                    a3 = agath[:].rearrange("p (s c) -> p s c", c=16)
                    aex = mpool.tile([P, S * 8], fp16, tag="aex")
                    nc.vector.tensor_tensor(
                        out=aex[:].rearrange("p (s h) -> p s h", h=8),
                        in0=g3[:, :, 128:136], in1=a3[:, :, d * 8:d * 8 + 8],
                        op=mybir.AluOpType.add)
                    lrl0 = mpool.tile([P, S * 8], fp16, tag="lrl0")
                    nc.vector.tensor_scalar(out=lrl0[:], in0=aex[:],
                                            scalar1=NEG_SLOPE, scalar2=None,
                                            op0=mybir.AluOpType.mult)
                    lrl = mpool.tile([P, S * 8], fp16, tag="lrl")
                    nc.vector.tensor_tensor(out=lrl[:], in0=aex[:], in1=lrl0[:],
                                            op=mybir.AluOpType.max)
                    ex = mpool.tile([P, S * 8], fp16, tag="ex")
                    nc.scalar.activation(out=ex[:], in_=lrl[:],
                                         func=mybir.ActivationFunctionType.Exp)

                    msg = opool.tile([P, S * P], fp16, tag="msg")
                    ex_b = ex[:].rearrange("p (s h o) -> p s h o", h=8, o=1)
                    nc.vector.tensor_tensor(
                        out=msg[:].rearrange("p (s h c) -> p s h c", h=8, c=16),
                        in0=g3[:, :, 0:128].rearrange("p s (h c) -> p s h c", c=16),
                        in1=ex_b.to_broadcast([P, S, 8, 16]),
                        op=mybir.AluOpType.mult)

                    pd = epsum.tile([P, 128], f32, tag="epsum")
                    pde = epsum.tile([P, 8], f32, tag="epsden")
                    for s in range(S):
                        nc.tensor.matmul(out=pd[:, :],
                                         lhsT=oneh[:, s * P:(s + 1) * P],
                                         rhs=msg[:, s * P:(s + 1) * P],
                                         start=(s == 0), stop=(s == S - 1))
                    for s in range(S):
                        nc.tensor.matmul(out=pde[:, :],
                                         lhsT=oneh[:, s * P:(s + 1) * P],
                                         rhs=ex[:, s * 8:(s + 1) * 8],
                                         start=(s == 0), stop=(s == S - 1))

                    den = mpool.tile([P, 8], f32, tag="den")
                    nc.vector.tensor_scalar(out=den[:], in0=pde[:, :],
                                            scalar1=1e-30, scalar2=None,
                                            op0=mybir.AluOpType.add)
                    rec = mpool.tile([P, 8], f32, tag="rec")
                    nc.vector.reciprocal(out=rec[:], in_=den[:])
                    od = dpool.tile([P, 128], f32, tag="od")
                    rec_b = rec[:].rearrange("p (h o) -> p h o", o=1)
                    nc.vector.tensor_tensor(
                        out=od[:].rearrange("p (h c) -> p h c", c=16),
                        in0=pd[:, :].rearrange("p (h c) -> p h c", c=16),
                        in1=rec_b.to_broadcast([P, 8, 16]),
                        op=mybir.AluOpType.mult)
                    outs_d.append(od)

                osum = dpool.tile([P, 128], f32, tag="osum")
                nc.vector.tensor_tensor(out=osum[:], in0=outs_d[0][:],
                                        in1=outs_d[1][:], op=mybir.AluOpType.add)
                ofin = dpool.tile([P, 128], f32, tag="ofin")
                nc.vector.tensor_tensor(out=ofin[:], in0=osum[:], in1=bias_s[:],
                                        op=mybir.AluOpType.add)
                nc.sync.dma_start(out=p_out[b * P:(b + 1) * P, :], in_=ofin[:])

    nc.compile()
    ctx.close()

    shared = {"xT": xT, "wcat": wcat, "iota": iota, "sent": sent, "bias": bias}
    in_maps = [dict(shared, einfo=einfo[k], ldst=ldsta[k]) for k in range(n_cores)]
    _LAST["nc"] = nc
    _LAST["in_maps"] = in_maps
    _LAST["n_cores"] = n_cores
    res = run_bass_kernel_spmd(nc, in_maps, list(range(n_cores)))
    full = np.concatenate([res.results[k]["out"] for k in range(n_cores)], axis=0)
    return full[:n_nodes].astype(np.float32)


def kernel(x, ei, W_in, a_src_in, a_dst_in, b_in, W_out, a_src_out, a_dst_out,
           b_out):
    x = np.asarray(x, np.float32)
    ei = np.asarray(ei, np.int32)
    return _build_and_run(
        x, ei,
        np.asarray(W_in, np.float32), np.asarray(a_src_in, np.float32),
        np.asarray(a_dst_in, np.float32), np.asarray(b_in, np.float32),
        np.asarray(W_out, np.float32), np.asarray(a_src_out, np.float32),
        np.asarray(a_dst_out, np.float32), np.asarray(b_out, np.float32),
        n_nodes=x.shape[0], n_edges=ei.shape[1])



# revision 2
# speedup vs baseline: 7.3629x; 7.3629x over previous
"""GAT DirSeq Trainium2 kernel, v3.

Key difference vs v2 (baseline): the edge phase uses a few large
`dma_gather` instructions (int16-indexed, 256B rows, 4-way sharded tables)
instead of thousands of small `indirect_dma_start` calls. SWDGE has ~1us
fixed cost per instruction and ~0.34ns per descriptor, so instruction count
is everything.

Layout:
  - Nodes scrambled: node r -> k = (r%128)*NB + r//128; shard t = (r%128)//32,
    within-shard row16 = ((r%128)%32)*NB + r//128 (< 32768 -> int16 ok).
  - T0/T1 tables [4*(RSH+1), 128] fp16: h_in / h_out rows (256B, last row of
    each shard = zero sentinel). Node-phase stores are contiguous per
    partition-group.
  - alpha_src is recomputed per edge from the gathered h row via an on-chip
    dot with a_src (mult + reduce), so it needs no table.
  - alpha_dst comes from a per-core compact table aK [NBLK_C*128+1, 128]
    (cols 0:16 = [adst_in|adst_out]) indexed by block-local key id, gathered
    with one dma_gather per group; filled by a mini node-phase over the
    core's own x slice (per-core x_own parameter keeps the program SPMD).
  - Edge phase: destination blocks grouped G at a time; per (group, dir):
    4 h-gathers (one per source shard) + 1 aK-gather; softmax as
    unnormalized weighted sums; scatter via one-hot matmuls with per-block
    masked labels (shared boundary columns are masked by label 999).
"""

import math
import numpy as np

N = 100000
E = 800000
D_IN = 128
HEADS = 8
C = 16
NEG_SLOPE = 0.2
P = 128
NSH = 4
_LAST = {}


# ------------------------------------------------------------------ host prep
def _prep(ei, n_nodes, n_cores, G):
    """Build per-core gather index streams, label arrays and static metadata."""
    NBLK_C = int(math.ceil(n_nodes / P / n_cores))
    NB = NBLK_C * n_cores
    NPAD = NB * P
    RSH = 32 * NB              # rows per shard (int16-addressable)
    NG = NBLK_C // G           # groups per core
    assert NBLK_C % G == 0 and NB % NSH == 0 and RSH < 32768

    AKS = NBLK_C * P           # aK sentinel row (per-core compact table)
    meta = {"NBLK_C": NBLK_C, "NB": NB, "NPAD": NPAD, "RSH": RSH, "NG": NG,
            "G": G, "AKS": AKS}

    src, dst = ei[0].astype(np.int64), ei[1].astype(np.int64)
    dirs = []
    for d in range(2):
        key = dst if d == 0 else src   # grouping (destination) node
        oth = src if d == 0 else dst   # message source node
        kb = key >> 7
        core = kb // NBLK_C
        qb = kb % NBLK_C
        g = qb // G
        q = qb % G
        klm = key & 127
        t = (oth & 127) >> 5
        row16 = ((oth & 127) & 31) * NB + (oth >> 7)
        kloc = qb * P + klm
        seg = ((core * NG + g) * NSH + t)
        order = np.lexsort((qb, seg))
        dirs.append(dict(core=core[order], g=g[order], q=q[order],
                         klm=klm[order], t=t[order], row16=row16[order],
                         kloc=kloc[order], seg=seg[order], qb=qb[order]))

    # static segment lengths (max over cores, padded to 128)
    seglen = np.zeros((2, NG, NSH), np.int64)
    for d in range(2):
        cnt = np.bincount(dirs[d]["seg"] % (NG * NSH) +
                          dirs[d]["core"] * (NG * NSH),
                          minlength=n_cores * NG * NSH
                          ).reshape(n_cores, NG, NSH)
        m = cnt.max(axis=0)
        seglen[d] = np.maximum(128, ((m + 127) // 128) * 128)

    colbase = np.zeros((2, NG, NSH), np.int64)   # column offset inside group
    SG = np.zeros((2, NG), np.int64)
    for d in range(2):
        for g in range(NG):
            cb = 0
            for t in range(NSH):
                colbase[d, g, t] = cb
                cb += seglen[d, g, t] // 128
            SG[d, g] = cb
    SGMAX = int(SG.max())

    # per-edge slot position within its group's stream
    for d in range(2):
        dd = dirs[d]
        segid = dd["core"] * (NG * NSH) + dd["seg"] % (NG * NSH)
        # rank within segment
        starts = np.searchsorted(segid, np.arange(n_cores * NG * NSH))
        rank = np.arange(segid.size) - starts[segid]
        dd["slot"] = colbase[d, dd["g"], dd["t"]] * 128 + rank
        dd["col"] = dd["slot"] >> 7

    # per-(d,g,q,t) column ranges (union over cores) and label offsets.
    # A block's edges form one contiguous run per source shard segment.
    c0 = np.zeros((2, NG, G, NSH), np.int64)
    nc_ = np.zeros((2, NG, G, NSH), np.int64)
    for d in range(2):
        dd = dirs[d]
        gqt = (dd["g"] * G + dd["q"]) * NSH + dd["t"]
        cmin = np.full(NG * G * NSH, 1 << 30, np.int64)
        cmax = np.full(NG * G * NSH, -1, np.int64)
        np.minimum.at(cmin, gqt, dd["col"])
        np.maximum.at(cmax, gqt, dd["col"])
        has = cmax >= 0
        c0[d][has.reshape(NG, G, NSH)] = cmin[has]
        nc_[d].reshape(-1)[has] = (cmax - cmin + 1)[has]
    # blocks with zero edges anywhere: give one inert column
    tot = nc_.sum(axis=3)
    for d in range(2):
        for g in range(NG):
            for q in range(G):
                if tot[d, g, q] == 0:
                    nc_[d, g, q, 0] = 1
    NCMAX = int(nc_.sum(axis=3).max())
    lblofs = np.zeros((2, NG, G), np.int64)       # label col offset per block
    tofsw = np.zeros((2, NG, G, NSH), np.int64)   # within-block per-shard ofs
    acc = 0
    for d in range(2):
        for g in range(NG):
            for q in range(G):
                lblofs[d, g, q] = acc
                w = 0
                for t in range(NSH):
                    tofsw[d, g, q, t] = w
                    w += nc_[d, g, q, t]
                acc += w
    LCOLS = int(acc)

    # group stream offsets (common to all cores)
    gofs = np.zeros((2, NG), np.int64)        # in slots
    itofs = np.zeros((2, NG, NSH), np.int64)  # idxT col offsets
    ikofs = np.zeros((2, NG), np.int64)       # idxK col offsets
    L = np.zeros(2, np.int64)
    ti, ki = 0, 0
    for d in range(2):
        o = 0
        for g in range(NG):
            gofs[d, g] = o
            for t in range(NSH):
                itofs[d, g, t] = ti
                ti += seglen[d, g, t] // 16
            ikofs[d, g] = ki
            ki += (SG[d, g] * 128) // 16
            o += SG[d, g] * 128
        L[d] = o
    TCOLS, KCOLS = int(ti), int(ki)

    # per-core streams
    idxT = np.full((n_cores, TCOLS * 16), RSH, np.int16)
    idxK = np.full((n_cores, KCOLS * 16), AKS, np.int16)
    lbl = np.full((n_cores, P, LCOLS), 999.0, np.float16)
    for d in range(2):
        dd = dirs[d]
        # stream position of each edge inside its core's direction stream
        spos = gofs[d, dd["g"]] + dd["slot"]
        # idxT: wrapped per segment; compute wrapped linear position:
        # within segment (g,t): j = slot - colbase*128 ... wrapped [16, len/16]
        j = dd["slot"] - colbase[d, dd["g"], dd["t"]] * 128
        wrap = (itofs[d, dd["g"], dd["t"]] + j // 16) * 16 + (j % 16)
        jk = dd["slot"]
        wrapk = (ikofs[d, dd["g"]] + jk // 16) * 16 + (jk % 16)
        pcol = (lblofs[d, dd["g"], dd["q"]] +
                tofsw[d, dd["g"], dd["q"], dd["t"]] +
                dd["col"] - c0[d, dd["g"], dd["q"], dd["t"]])
        for k in range(n_cores):
            m = dd["core"] == k
            idxT[k][wrap[m]] = dd["row16"][m]
            idxK[k][wrapk[m]] = dd["kloc"][m]
            lbl[k][dd["slot"][m] & 127, pcol[m]] = dd["klm"][m].astype(np.float16)
    # [16, cols] wrap -> replicate to 128 partitions
    idxT = np.ascontiguousarray(
        np.tile(idxT.reshape(n_cores, TCOLS, 16).transpose(0, 2, 1), (1, 8, 1)))
    idxK = np.ascontiguousarray(
        np.tile(idxK.reshape(n_cores, KCOLS, 16).transpose(0, 2, 1), (1, 8, 1)))

    meta.update(seglen=seglen, colbase=colbase, SG=SG, SGMAX=SGMAX, gofs=gofs,
                itofs=itofs, ikofs=ikofs, c0=c0, ncols=nc_, NCMAX=NCMAX,
                lblofs=lblofs, tofsw=tofsw, LCOLS=LCOLS, TCOLS=TCOLS,
                KCOLS=KCOLS)
    return meta, idxT, idxK, lbl


def _build(meta, n_cores, compile_only=True, build_stage=4):
    import concourse.bacc as bacc
    import concourse.mybir as mybir
    import concourse.tile as tile

    fp16 = mybir.dt.float16
    f32 = mybir.dt.float32
    i16 = mybir.dt.int16

    NBLK_C, NB, NPAD, RSH = meta["NBLK_C"], meta["NB"], meta["NPAD"], meta["RSH"]
    NG, G, AKS = meta["NG"], meta["G"], meta["AKS"]
    SGMAX, NCMAX, LCOLS = meta["SGMAX"], meta["NCMAX"], meta["LCOLS"]
    TCOLS, KCOLS = meta["TCOLS"], meta["KCOLS"]
    seglen, colbase, SG = meta["seglen"], meta["colbase"], meta["SG"]
    gofs, itofs, ikofs = meta["gofs"], meta["itofs"], meta["ikofs"]
    c0a, ncolsa, lblofs = meta["c0"], meta["ncols"], meta["lblofs"]
    RSHT = RSH + 1

    nc = bacc.Bacc(None, target_bir_lowering=False, debug=False,
                   num_swdge_queues=4)

    p_xT = nc.declare_dram_parameter("xT", [P, NPAD], fp16, isOutput=False)
    p_xo = nc.declare_dram_parameter("xo", [P, NBLK_C * P], fp16, isOutput=False)
    p_wcat = nc.declare_dram_parameter("wcat", [P, 256], fp16, isOutput=False)
    p_wdst = nc.declare_dram_parameter("wdst", [P, 16], fp16, isOutput=False)
    p_asrc = nc.declare_dram_parameter("asrcc", [P, 256], fp16, isOutput=False)
    p_iota = nc.declare_dram_parameter("iota", [P, NCMAX * P], fp16, isOutput=False)
    p_bias = nc.declare_dram_parameter("bias", [P, 128], f32, isOutput=False)
    p_zrow = nc.declare_dram_parameter("zrow", [1, 128], fp16, isOutput=False)
    p_nrow = nc.declare_dram_parameter("nrow", [1, 128], fp16, isOutput=False)
    p_idxT = nc.declare_dram_parameter("idxT", [P, TCOLS], i16, isOutput=False)
    p_idxK = nc.declare_dram_parameter("idxK", [P, KCOLS], i16, isOutput=False)
    p_lbl = nc.declare_dram_parameter("lbl", [P, LCOLS], fp16, isOutput=False)
    p_out = nc.declare_dram_parameter("out", [NBLK_C * P, 128], f32, isOutput=True)

    T = [nc.dram_tensor("T0", [NSH * RSHT, 128], fp16),
         nc.dram_tensor("T1", [NSH * RSHT, 128], fp16)]
    aK = nc.dram_tensor("aK", [NBLK_C * P + 1, 128], fp16)

    NBAT = 8

    with tile.TileContext(nc) as tc:
        with (
            tc.tile_pool(name="const", bufs=1) as cpool,
            tc.tile_pool(name="gh", bufs=2) as gpool,
            tc.tile_pool(name="agh", bufs=2) as apool,
            tc.tile_pool(name="tmp", bufs=1) as tpool,
            tc.tile_pool(name="msg", bufs=2) as mpool,
            tc.tile_pool(name="sm", bufs=2) as smpool,
            tc.tile_pool(name="oneh", bufs=3) as opool,
            tc.tile_pool(name="idx", bufs=2) as ipool,
            tc.tile_pool(name="epi", bufs=2) as dpool,
            tc.tile_pool(name="odp", bufs=2) as odpool,
            tc.tile_pool(name="pd", bufs=4, space="PSUM") as ppool,
        ):
            wcat_s = cpool.tile([P, 256], fp16)
            nc.sync.dma_start(out=wcat_s[:], in_=p_wcat[:])
            wdst_s = cpool.tile([P, 16], fp16)
            nc.sync.dma_start(out=wdst_s[:], in_=p_wdst[:])
            asrc_s = cpool.tile([P, 256], fp16)
            nc.sync.dma_start(out=asrc_s[:], in_=p_asrc[:])
            iota_s = cpool.tile([P, NCMAX * P], fp16)
            nc.sync.dma_start(out=iota_s[:], in_=p_iota[:])
            bias_s = cpool.tile([P, 128], f32)
            nc.sync.dma_start(out=bias_s[:], in_=p_bias[:])
            lbl_s = cpool.tile([P, LCOLS], fp16)
            nc.sync.dma_start(out=lbl_s[:], in_=p_lbl[:])

            # sentinel rows
            for d in range(2):
                for t in range(NSH):
                    nc.sync.dma_start(out=T[d][t * RSHT + RSH:t * RSHT + RSHT, :],
                                      in_=p_zrow[:])
            nc.sync.dma_start(out=aK[NBLK_C * P:NBLK_C * P + 1, :], in_=p_nrow[:])

            # ---------------- node phase: h tables ----------------
            with (
                tc.tile_pool(name="xc", bufs=2) as xpool,
                tc.tile_pool(name="stage", bufs=2) as spool,
                tc.tile_pool(name="nps", bufs=2, space="PSUM") as npsum,
            ):
                for g0 in range(0, NB, NBAT):
                    nb = min(NBAT, NB - g0)
                    xc = xpool.tile([P, NBAT * P], fp16, tag="xc")
                    nc.sync.dma_start(out=xc[:, 0:nb * P],
                                      in_=p_xT[:, g0 * P:(g0 + nb) * P])
                    stage = spool.tile([P, NBAT * 256], fp16, tag="stage")
                    for j in range(nb):
                        ps = npsum.tile([P, 256], f32, tag="nps")
                        nc.tensor.matmul(out=ps[:], lhsT=xc[:, j * P:(j + 1) * P],
                                         rhs=wcat_s[:], start=True, stop=True)
                        dstg = stage[:, j * 256:(j + 1) * 256]
                        if j % 2 == 0:
                            nc.scalar.copy(out=dstg, in_=ps[:])
                        else:
                            nc.vector.tensor_copy(out=dstg, in_=ps[:])
                    st3 = stage[:].rearrange("p (j c) -> p j c", c=256)
                    for d in range(2):
                        for t in range(NSH):
                            dv = T[d][t * RSHT:t * RSHT + RSH, :].rearrange(
                                "(q n) c -> q n c", n=NB)[:, g0:g0 + nb, :]
                            nc.sync.dma_start(
                                out=dv,
                                in_=st3[32 * t:32 * (t + 1), 0:nb,
                                        d * 128:(d + 1) * 128])

                # ---------------- node phase: aK (own blocks) ----------------
                NB2 = 7
                for b0 in range(0, NBLK_C, NB2):
                    nb = min(NB2, NBLK_C - b0)
                    xo = xpool.tile([P, NB2 * P], fp16, tag="xo")
                    nc.sync.dma_start(out=xo[:, 0:nb * P],
                                      in_=p_xo[:, b0 * P:(b0 + nb) * P])
                    stga = spool.tile([P, NB2 * 16], fp16, tag="stga")
                    for j in range(nb):
                        psa = npsum.tile([P, 16], f32, tag="npsa")
                        nc.tensor.matmul(out=psa[:], lhsT=xo[:, j * P:(j + 1) * P],
                                         rhs=wdst_s[:], start=True, stop=True)
                        if j % 2 == 0:
                            nc.scalar.copy(out=stga[:, j * 16:(j + 1) * 16], in_=psa[:])
                        else:
                            nc.vector.tensor_copy(out=stga[:, j * 16:(j + 1) * 16],
                                                  in_=psa[:])
                    av = aK[b0 * P:(b0 + nb) * P, 0:16].rearrange(
                        "(j p) c -> p j c", p=P)
                    nc.sync.dma_start(
                        out=av, in_=stga[:].rearrange("p (j c) -> p j c", c=16)[:, 0:nb, :])

            # ---------------- edge phase ----------------
            if build_stage == 1:
                for b in range(NBLK_C):
                    ofin = dpool.tile([P, 128], f32, tag="ofin")
                    nc.vector.tensor_copy(out=ofin[:], in_=bias_s[:])
                    nc.sync.dma_start(out=p_out[b * P:(b + 1) * P, :], in_=ofin[:])
            for g in (range(NG) if build_stage >= 2 or build_stage in (20, 21, 22) else []):
                ods = {}
                for d in range(2):
                    Sg = int(SG[d, g])
                    gTcols = Sg * 8
                    idxt = ipool.tile([P, SGMAX * 8], i16, tag="idxt")
                    nc.sync.dma_start(
                        out=idxt[:, 0:gTcols],
                        in_=p_idxT[:, int(itofs[d, g, 0]):int(itofs[d, g, 0]) + gTcols])
                    idxk = ipool.tile([P, SGMAX * 8], i16, tag="idxk")
                    nc.sync.dma_start(
                        out=idxk[:, 0:gTcols],
                        in_=p_idxK[:, int(ikofs[d, g]):int(ikofs[d, g]) + gTcols])

                    gh = gpool.tile([P, SGMAX * P], fp16, tag="gh")
                    gh3 = gh[:].rearrange("p (s c) -> p s c", c=P)
                    if build_stage == 21:
                        nc.vector.tensor_scalar(
                            out=gh[:], in0=iota_s[:, 0:1].to_broadcast([P, SGMAX * P]),
                            scalar1=0.0, scalar2=None, op0=mybir.AluOpType.mult)
                    for t in (range(NSH) if build_stage != 21 else []):
                        sl = int(seglen[d, g, t])
                        cb = int(colbase[d, g, t])
                        io = int(itofs[d, g, t] - itofs[d, g, 0])
                        base = 0 if build_stage == 22 else t * RSHT
                        nc.gpsimd.dma_gather(
                            out_ap=gh3[:, cb:cb + sl // 128, :],
                            in_ap=T[d][base:base + RSHT, :],
                            idxs_ap=idxt[:, io:io + sl // 16],
                            num_idxs=sl, num_idxs_reg=sl, elem_size=P,
                            single_packet=False, queue_num=t % 4)
                    agh = apool.tile([P, SGMAX * P], fp16, tag="agh")
                    agh3 = agh[:].rearrange("p (s c) -> p s c", c=P)
                    if build_stage != 20:
                        nc.gpsimd.dma_gather(
                            out_ap=agh3[:, 0:Sg, :],
                            in_ap=aK[0:NBLK_C * P + 1, :],
                            idxs_ap=idxk[:, 0:gTcols],
                            num_idxs=Sg * 128, num_idxs_reg=Sg * 128, elem_size=P,
                            single_packet=False, queue_num=(d + g) % 4)
                    else:
                        nc.vector.tensor_scalar(out=agh[:], in0=gh[:],
                                                scalar1=0.0, scalar2=None,
                                                op0=mybir.AluOpType.mult)

                    if build_stage in (2, 20, 21, 22):
                        od2 = odpool.tile([P, 128], f32, tag=f"od{d}_0s2")
                        nc.vector.tensor_copy(out=od2[:], in_=gh[:, 0:128])
                        od2b = odpool.tile([P, 128], f32, tag=f"od{d}_1s2")
                        nc.vector.tensor_copy(out=od2b[:], in_=agh[:, 0:128])
                        for q in range(G):
                            ods[(d, q)] = od2 if q % 2 == 0 else od2b
                        continue
                    # alpha_src = <h, a_src_d> per head
                    tmp = tpool.tile([P, SGMAX * P], fp16, tag="tmp")
                    nc.vector.tensor_tensor(
                        out=tmp[:, 0:Sg * P].rearrange("p (s h c) -> p s h c",
                                                       h=HEADS, c=C),
                        in0=gh[:, 0:Sg * P].rearrange("p (s h c) -> p s h c",
                                                      h=HEADS, c=C),
                        in1=asrc_s[:, d * 128:(d + 1) * 128].rearrange(
                            "p (o h c) -> p o h c", o=1, h=HEADS, c=C
                        ).to_broadcast([P, Sg, HEADS, C]),
                        op=mybir.AluOpType.mult)
                    asr = smpool.tile([P, SGMAX * 8], f32, tag="asr")
                    nc.vector.tensor_reduce(
                        out=asr[:, 0:Sg * 8],
                        in_=tmp[:, 0:Sg * P].rearrange("p (sh c) -> p sh c", c=C),
                        axis=mybir.AxisListType.X, op=mybir.AluOpType.add)

                    aex = smpool.tile([P, SGMAX * 8], fp16, tag="aex")
                    nc.vector.tensor_tensor(
                        out=aex[:, 0:Sg * 8].rearrange("p (s h) -> p s h", h=8),
                        in0=asr[:, 0:Sg * 8].rearrange("p (s h) -> p s h", h=8),
                        in1=agh3[:, 0:Sg, d * 8:(d + 1) * 8],
                        op=mybir.AluOpType.add)
                    lrl0 = smpool.tile([P, SGMAX * 8], fp16, tag="lrl0")
                    nc.vector.tensor_scalar(out=lrl0[:, 0:Sg * 8],
                                            in0=aex[:, 0:Sg * 8],
                                            scalar1=NEG_SLOPE, scalar2=None,
                                            op0=mybir.AluOpType.mult)
                    lrl = smpool.tile([P, SGMAX * 8], fp16, tag="lrl")
                    nc.vector.tensor_tensor(out=lrl[:, 0:Sg * 8],
                                            in0=aex[:, 0:Sg * 8],
                                            in1=lrl0[:, 0:Sg * 8],
                                            op=mybir.AluOpType.max)
                    ex = smpool.tile([P, SGMAX * 8], fp16, tag="ex")
                    nc.scalar.activation(out=ex[:, 0:Sg * 8], in_=lrl[:, 0:Sg * 8],
                                         func=mybir.ActivationFunctionType.Exp)

                    # msg slots: [ex*h (128) | ex (8)]
                    msg = mpool.tile([P, SGMAX * 136], fp16, tag="msg")
                    msg3 = msg[:].rearrange("p (s c) -> p s c", c=136)
                    nc.vector.tensor_tensor(
                        out=msg3[:, 0:Sg, 0:128].rearrange("p s (h c) -> p s h c",
                                                           c=C),
                        in0=gh3[:, 0:Sg, :].rearrange("p s (h c) -> p s h c", c=C),
                        in1=ex[:, 0:Sg * 8].rearrange("p (s h o) -> p s h o",
                                                      h=8, o=1
                                                      ).to_broadcast([P, Sg, 8, C]),
                        op=mybir.AluOpType.mult)
                    nc.vector.tensor_copy(
                        out=msg3[:, 0:Sg, 128:136],
                        in_=ex[:, 0:Sg * 8].rearrange("p (s h) -> p s h", h=8))

                    if build_stage == 3:
                        for q in range(G):
                            od3 = odpool.tile([P, 128], f32, tag=f"od{d}_{q}")
                            nc.vector.tensor_copy(out=od3[:], in_=msg[:, 0:128])
                            ods[(d, q)] = od3
                        continue
                    for q in range(G):
                        ncq = int(ncolsa[d, g, q].sum())
                        lo = int(lblofs[d, g, q])
                        oneh = opool.tile([P, NCMAX * P], fp16, tag="oneh")
                        nc.vector.tensor_tensor(
                            out=oneh[:, 0:ncq * P].rearrange("p (n e) -> p n e",
                                                             e=P),
                            in0=lbl_s[:, lo:lo + ncq].rearrange(
                                "p (n o) -> p n o", o=1).to_broadcast([P, ncq, P]),
                            in1=iota_s[:, 0:ncq * P].rearrange("p (n e) -> p n e",
                                                               e=P),
                            op=mybir.AluOpType.is_equal)
                        pd = ppool.tile([P, 136], f32, tag="pd")
                        i = 0
                        for t in range(NSH):
                            for w in range(int(ncolsa[d, g, q, t])):
                                cc = int(c0a[d, g, q, t]) + w
                                nc.tensor.matmul(
                                    out=pd[:],
                                    lhsT=oneh[:, i * P:(i + 1) * P],
                                    rhs=msg[:, cc * 136:(cc + 1) * 136],
                                    start=(i == 0), stop=(i == ncq - 1))
                                i += 1
                        den = dpool.tile([P, 8], f32, tag="den")
                        nc.vector.tensor_scalar(out=den[:], in0=pd[:, 128:136],
                                                scalar1=1e-30, scalar2=None,
                                                op0=mybir.AluOpType.add)
                        rec = dpool.tile([P, 8], f32, tag="rec")
                        nc.vector.reciprocal(out=rec[:], in_=den[:])
                        od = odpool.tile([P, 128], f32, tag=f"od{d}_{q}")
                        nc.vector.tensor_tensor(
                            out=od[:].rearrange("p (h c) -> p h c", c=C),
                            in0=pd[:, 0:128].rearrange("p (h c) -> p h c", c=C),
                            in1=rec[:].rearrange("p (h o) -> p h o", o=1
                                                 ).to_broadcast([P, 8, C]),
                            op=mybir.AluOpType.mult)
                        ods[(d, q)] = od

                for q in range(G):
                    osum = dpool.tile([P, 128], f32, tag="osum")
                    nc.vector.tensor_tensor(out=osum[:], in0=ods[(0, q)][:],
                                            in1=ods[(1, q)][:],
                                            op=mybir.AluOpType.add)
                    ofin = dpool.tile([P, 128], f32, tag="ofin")
                    nc.vector.tensor_tensor(out=ofin[:], in0=osum[:], in1=bias_s[:],
                                            op=mybir.AluOpType.add)
                    b = g * G + q
                    nc.sync.dma_start(out=p_out[b * P:(b + 1) * P, :], in_=ofin[:])

    nc.compile()
    return nc


def _host_inputs(meta, x, W_in, a_src_in, a_dst_in, b_in, W_out, a_src_out,
                 a_dst_out, b_out, idxT, idxK, lbl, n_nodes, n_cores):
    NB, NPAD, NBLK_C, NCMAX = meta["NB"], meta["NPAD"], meta["NBLK_C"], meta["NCMAX"]
    Vdst_in = np.stack([W_in[:, h * C:(h + 1) * C] @ a_dst_in[h]
                        for h in range(HEADS)], 1)   # [D_IN, HEADS]
    Vdst_out = np.stack([W_out[:, h * C:(h + 1) * C] @ a_dst_out[h]
                         for h in range(HEADS)], 1)
    wcat = np.concatenate([W_in, W_out], axis=1).astype(np.float16)
    wdst = np.concatenate([Vdst_in, Vdst_out], axis=1).astype(np.float16)
    asrcc = np.tile(np.concatenate([a_src_in.reshape(-1), a_src_out.reshape(-1)]
                                   ).astype(np.float16)[None, :], (P, 1))
    xT = np.zeros((D_IN, NPAD), np.float16)
    xT[:, :n_nodes] = x.T.astype(np.float16)
    iota = np.tile(np.arange(P, dtype=np.float16), (P, NCMAX)).reshape(P, NCMAX * P)
    bias = np.tile((b_in + b_out).astype(np.float32)[None, :], (P, 1))
    zrow = np.zeros((1, 128), np.float16)
    nrow = np.full((1, 128), -30000.0, np.float16)

    shared = dict(xT=xT, wcat=wcat, wdst=wdst, asrcc=asrcc, iota=iota,
                  bias=bias, zrow=zrow, nrow=nrow)
    in_maps = []
    for k in range(n_cores):
        xo = xT[:, k * NBLK_C * P:(k + 1) * NBLK_C * P]
        in_maps.append(dict(shared, xo=np.ascontiguousarray(xo),
                            idxT=idxT[k], idxK=idxK[k], lbl=lbl[k]))
    return in_maps


def kernel(x, ei, W_in, a_src_in, a_dst_in, b_in, W_out, a_src_out, a_dst_out,
           b_out, n_cores=8, G=7):
    from concourse.bass_utils import run_bass_kernel_spmd

    x = np.asarray(x, np.float32)
    ei = np.asarray(ei, np.int32)
    n_nodes = x.shape[0]
    meta, idxT, idxK, lbl = _prep(ei, n_nodes, n_cores, G)
    nc = _build(meta, n_cores)
    in_maps = _host_inputs(meta, x,
                           np.asarray(W_in, np.float32),
                           np.asarray(a_src_in, np.float32),
                           np.asarray(a_dst_in, np.float32),
                           np.asarray(b_in, np.float32),
                           np.asarray(W_out, np.float32),
                           np.asarray(a_src_out, np.float32),
                           np.asarray(a_dst_out, np.float32),
                           np.asarray(b_out, np.float32),
                           idxT, idxK, lbl, n_nodes, n_cores)
    _LAST.update(nc=nc, in_maps=in_maps, n_cores=n_cores, meta=meta)
    res = run_bass_kernel_spmd(nc, in_maps, list(range(n_cores)))
    full = np.concatenate([res.results[k]["out"] for k in range(n_cores)], axis=0)
    return full[:n_nodes].astype(np.float32)


# revision 3
# speedup vs baseline: 10.5606x; 1.4343x over previous
"""GAT DirSeq Trainium2 kernel, v3.

Key difference vs v2 (baseline): the edge phase uses a few large
`dma_gather` instructions (int16-indexed, 256B rows, 4-way sharded tables)
instead of thousands of small `indirect_dma_start` calls. SWDGE has ~1us
fixed cost per instruction and ~0.34ns per descriptor, so instruction count
is everything.

Layout:
  - Nodes scrambled: node r -> k = (r%128)*NB + r//128; shard t = (r%128)//32,
    within-shard row16 = ((r%128)%32)*NB + r//128 (< 32768 -> int16 ok).
  - T0/T1 tables [4*(RSH+1), 128] fp16: h_in / h_out rows (256B, last row of
    each shard = zero sentinel). Node-phase stores are contiguous per
    partition-group.
  - alpha_src is recomputed per edge from the gathered h row via an on-chip
    dot with a_src (mult + reduce), so it needs no table.
  - alpha_dst comes from a per-core compact table aK [NBLK_C*128+1, 128]
    (cols 0:16 = [adst_in|adst_out]) indexed by block-local key id, gathered
    with one dma_gather per group; filled by a mini node-phase over the
    core's own x slice (per-core x_own parameter keeps the program SPMD).
  - Edge phase: destination blocks grouped G at a time; per (group, dir):
    4 h-gathers (one per source shard) + 1 aK-gather; softmax as
    unnormalized weighted sums; scatter via one-hot matmuls with per-block
    masked labels (shared boundary columns are masked by label 999).
"""

import math
import numpy as np

N = 100000
E = 800000
D_IN = 128
HEADS = 8
C = 16
NEG_SLOPE = 0.2
P = 128
NSH = 4
_LAST = {}


# ------------------------------------------------------------------ host prep
def _prep(ei, n_nodes, n_cores, G):
    """Build per-core gather index streams, label arrays and static metadata."""
    NBLK_C = int(math.ceil(n_nodes / P / n_cores))
    NB = NBLK_C * n_cores
    NPAD = NB * P
    RSH = 32 * NB              # rows per shard (int16-addressable)
    NG = NBLK_C // G           # groups per core
    assert NBLK_C % G == 0 and NB % NSH == 0 and RSH < 32768

    AKS = NBLK_C * P           # aK sentinel row (per-core compact table)
    meta = {"NBLK_C": NBLK_C, "NB": NB, "NPAD": NPAD, "RSH": RSH, "NG": NG,
            "G": G, "AKS": AKS}

    src, dst = ei[0].astype(np.int64), ei[1].astype(np.int64)
    dirs = []
    for d in range(2):
        key = dst if d == 0 else src   # grouping (destination) node
        oth = src if d == 0 else dst   # message source node
        kb = key >> 7
        core = kb // NBLK_C
        qb = kb % NBLK_C
        g = qb // G
        q = qb % G
        klm = key & 127
        t = (oth & 127) >> 5
        row16 = ((oth & 127) & 31) * NB + (oth >> 7)
        kloc = qb * P + klm
        seg = ((core * NG + g) * NSH + t)
        order = np.lexsort((qb, seg))
        dirs.append(dict(core=core[order], g=g[order], q=q[order],
                         klm=klm[order], t=t[order], row16=row16[order],
                         kloc=kloc[order], seg=seg[order], qb=qb[order]))

    # static segment lengths (max over cores, padded to 128)
    seglen = np.zeros((2, NG, NSH), np.int64)
    for d in range(2):
        cnt = np.bincount(dirs[d]["seg"] % (NG * NSH) +
                          dirs[d]["core"] * (NG * NSH),
                          minlength=n_cores * NG * NSH
                          ).reshape(n_cores, NG, NSH)
        m = cnt.max(axis=0)
        seglen[d] = np.maximum(128, ((m + 127) // 128) * 128)

    colbase = np.zeros((2, NG, NSH), np.int64)   # column offset inside group
    SG = np.zeros((2, NG), np.int64)
    for d in range(2):
        for g in range(NG):
            cb = 0
            for t in range(NSH):
                colbase[d, g, t] = cb
                cb += seglen[d, g, t] // 128
            SG[d, g] = cb
    SGMAX = int(SG.max())

    # per-edge slot position within its group's stream
    for d in range(2):
        dd = dirs[d]
        segid = dd["core"] * (NG * NSH) + dd["seg"] % (NG * NSH)
        # rank within segment
        starts = np.searchsorted(segid, np.arange(n_cores * NG * NSH))
        rank = np.arange(segid.size) - starts[segid]
        dd["slot"] = colbase[d, dd["g"], dd["t"]] * 128 + rank
        dd["col"] = dd["slot"] >> 7

    # per-(d,g,q,t) column ranges (union over cores) and label offsets.
    # A block's edges form one contiguous run per source shard segment.
    c0 = np.zeros((2, NG, G, NSH), np.int64)
    nc_ = np.zeros((2, NG, G, NSH), np.int64)
    for d in range(2):
        dd = dirs[d]
        gqt = (dd["g"] * G + dd["q"]) * NSH + dd["t"]
        cmin = np.full(NG * G * NSH, 1 << 30, np.int64)
        cmax = np.full(NG * G * NSH, -1, np.int64)
        np.minimum.at(cmin, gqt, dd["col"])
        np.maximum.at(cmax, gqt, dd["col"])
        has = cmax >= 0
        c0[d][has.reshape(NG, G, NSH)] = cmin[has]
        nc_[d].reshape(-1)[has] = (cmax - cmin + 1)[has]
    # blocks with zero edges anywhere: give one inert column
    tot = nc_.sum(axis=3)
    for d in range(2):
        for g in range(NG):
            for q in range(G):
                if tot[d, g, q] == 0:
                    nc_[d, g, q, 0] = 1
    NCMAX = int(nc_.sum(axis=3).max())
    lblofs = np.zeros((2, NG, G), np.int64)       # label col offset per block
    tofsw = np.zeros((2, NG, G, NSH), np.int64)   # within-block per-shard ofs
    acc = 0
    for d in range(2):
        for g in range(NG):
            for q in range(G):
                lblofs[d, g, q] = acc
                w = 0
                for t in range(NSH):
                    tofsw[d, g, q, t] = w
                    w += nc_[d, g, q, t]
                acc += w
    LCOLS = int(acc)

    # group stream offsets (common to all cores)
    gofs = np.zeros((2, NG), np.int64)        # in slots
    itofs = np.zeros((2, NG, NSH), np.int64)  # idxT col offsets
    ikofs = np.zeros((2, NG), np.int64)       # idxK col offsets
    L = np.zeros(2, np.int64)
    ti, ki = 0, 0
    for d in range(2):
        o = 0
        for g in range(NG):
            gofs[d, g] = o
            for t in range(NSH):
                itofs[d, g, t] = ti
                ti += seglen[d, g, t] // 16
            ikofs[d, g] = ki
            ki += (SG[d, g] * 128) // 16
            o += SG[d, g] * 128
        L[d] = o
    TCOLS, KCOLS = int(ti), int(ki)

    # per-core streams
    idxT = np.full((n_cores, TCOLS * 16), RSH, np.int16)
    idxK = np.full((n_cores, KCOLS * 16), AKS, np.int16)
    lbl = np.full((n_cores, P, LCOLS), 999.0, np.float16)
    for d in range(2):
        dd = dirs[d]
        # stream position of each edge inside its core's direction stream
        spos = gofs[d, dd["g"]] + dd["slot"]
        # idxT: wrapped per segment; compute wrapped linear position:
        # within segment (g,t): j = slot - colbase*128 ... wrapped [16, len/16]
        j = dd["slot"] - colbase[d, dd["g"], dd["t"]] * 128
        wrap = (itofs[d, dd["g"], dd["t"]] + j // 16) * 16 + (j % 16)
        jk = dd["slot"]
        wrapk = (ikofs[d, dd["g"]] + jk // 16) * 16 + (jk % 16)
        pcol = (lblofs[d, dd["g"], dd["q"]] +
                tofsw[d, dd["g"], dd["q"], dd["t"]] +
                dd["col"] - c0[d, dd["g"], dd["q"], dd["t"]])
        for k in range(n_cores):
            m = dd["core"] == k
            idxT[k][wrap[m]] = dd["row16"][m]
            idxK[k][wrapk[m]] = dd["kloc"][m]
            lbl[k][dd["slot"][m] & 127, pcol[m]] = dd["klm"][m].astype(np.float16)
    # [16, cols] wrap -> replicate to 128 partitions
    idxT = np.ascontiguousarray(
        np.tile(idxT.reshape(n_cores, TCOLS, 16).transpose(0, 2, 1), (1, 8, 1)))
    idxK = np.ascontiguousarray(
        np.tile(idxK.reshape(n_cores, KCOLS, 16).transpose(0, 2, 1), (1, 8, 1)))

    meta.update(seglen=seglen, colbase=colbase, SG=SG, SGMAX=SGMAX, gofs=gofs,
                itofs=itofs, ikofs=ikofs, c0=c0, ncols=nc_, NCMAX=NCMAX,
                lblofs=lblofs, tofsw=tofsw, LCOLS=LCOLS, TCOLS=TCOLS,
                KCOLS=KCOLS)
    return meta, idxT, idxK, lbl


def _build(meta, n_cores, compile_only=True, build_stage=4):
    import concourse.bacc as bacc
    import concourse.mybir as mybir
    import concourse.tile as tile

    fp16 = mybir.dt.float16
    f32 = mybir.dt.float32
    i16 = mybir.dt.int16

    NBLK_C, NB, NPAD, RSH = meta["NBLK_C"], meta["NB"], meta["NPAD"], meta["RSH"]
    NG, G, AKS = meta["NG"], meta["G"], meta["AKS"]
    SGMAX, NCMAX, LCOLS = meta["SGMAX"], meta["NCMAX"], meta["LCOLS"]
    TCOLS, KCOLS = meta["TCOLS"], meta["KCOLS"]
    seglen, colbase, SG = meta["seglen"], meta["colbase"], meta["SG"]
    gofs, itofs, ikofs = meta["gofs"], meta["itofs"], meta["ikofs"]
    c0a, ncolsa, lblofs = meta["c0"], meta["ncols"], meta["lblofs"]
    RSHT = RSH + 1

    nc = bacc.Bacc(None, target_bir_lowering=False, debug=False,
                   num_swdge_queues=4)

    p_xT = nc.declare_dram_parameter("xT", [P, NPAD], fp16, isOutput=False)
    p_xo = nc.declare_dram_parameter("xo", [P, NBLK_C * P], fp16, isOutput=False)
    p_wcat = nc.declare_dram_parameter("wcat", [P, 256], fp16, isOutput=False)
    p_wdst = nc.declare_dram_parameter("wdst", [P, 16], fp16, isOutput=False)
    p_asrc = nc.declare_dram_parameter("asrcc", [P, 256], fp16, isOutput=False)
    p_iota = nc.declare_dram_parameter("iota", [P, NCMAX * P], fp16, isOutput=False)
    p_bias = nc.declare_dram_parameter("bias", [P, 128], f32, isOutput=False)
    p_zrow = nc.declare_dram_parameter("zrow", [1, 128], fp16, isOutput=False)
    p_nrow = nc.declare_dram_parameter("nrow", [1, 128], fp16, isOutput=False)
    p_idxT = nc.declare_dram_parameter("idxT", [P, TCOLS], i16, isOutput=False)
    p_idxK = nc.declare_dram_parameter("idxK", [P, KCOLS], i16, isOutput=False)
    p_lbl = nc.declare_dram_parameter("lbl", [P, LCOLS], fp16, isOutput=False)
    p_out = nc.declare_dram_parameter("out", [NBLK_C * P, 128], f32, isOutput=True)

    T = [nc.dram_tensor("T0", [NSH * RSHT, 128], fp16),
         nc.dram_tensor("T1", [NSH * RSHT, 128], fp16)]
    aK = nc.dram_tensor("aK", [NBLK_C * P + 1, 128], fp16)

    NBAT = 8

    with tile.TileContext(nc) as tc:
        with (
            tc.tile_pool(name="const", bufs=1) as cpool,
            tc.tile_pool(name="gh", bufs=2) as gpool,
            tc.tile_pool(name="agh", bufs=2) as apool,
            tc.tile_pool(name="tmp", bufs=1) as tpool,
            tc.tile_pool(name="msg", bufs=2) as mpool,
            tc.tile_pool(name="sm", bufs=2) as smpool,
            tc.tile_pool(name="oneh", bufs=3) as opool,
            tc.tile_pool(name="idx", bufs=2) as ipool,
            tc.tile_pool(name="epi", bufs=2) as dpool,
            tc.tile_pool(name="odp", bufs=2) as odpool,
            tc.tile_pool(name="pd", bufs=4, space="PSUM") as ppool,
        ):
            wcat_s = cpool.tile([P, 256], fp16)
            nc.sync.dma_start(out=wcat_s[:], in_=p_wcat[:])
            wdst_s = cpool.tile([P, 16], fp16)
            nc.sync.dma_start(out=wdst_s[:], in_=p_wdst[:])
            asrc_s = cpool.tile([P, 256], fp16)
            nc.sync.dma_start(out=asrc_s[:], in_=p_asrc[:])
            iota_s = cpool.tile([P, NCMAX * P], fp16)
            nc.sync.dma_start(out=iota_s[:], in_=p_iota[:])
            bias_s = cpool.tile([P, 128], f32)
            nc.sync.dma_start(out=bias_s[:], in_=p_bias[:])
            lbl_s = cpool.tile([P, LCOLS], fp16)
            nc.sync.dma_start(out=lbl_s[:], in_=p_lbl[:])

            # sentinel rows
            for d in range(2):
                for t in range(NSH):
                    nc.sync.dma_start(out=T[d][t * RSHT + RSH:t * RSHT + RSHT, :],
                                      in_=p_zrow[:])
            nc.sync.dma_start(out=aK[NBLK_C * P:NBLK_C * P + 1, :], in_=p_nrow[:])

            # ---------------- node phase: h tables ----------------
            with (
                tc.tile_pool(name="xc", bufs=2) as xpool,
                tc.tile_pool(name="stage", bufs=2) as spool,
                tc.tile_pool(name="nps", bufs=2, space="PSUM") as npsum,
            ):
                for g0 in range(0, NB, NBAT):
                    nb = min(NBAT, NB - g0)
                    xc = xpool.tile([P, NBAT * P], fp16, tag="xc")
                    nc.sync.dma_start(out=xc[:, 0:nb * P],
                                      in_=p_xT[:, g0 * P:(g0 + nb) * P])
                    stage = spool.tile([P, NBAT * 256], fp16, tag="stage")
                    for j in range(nb):
                        ps = npsum.tile([P, 256], f32, tag="nps")
                        nc.tensor.matmul(out=ps[:], lhsT=xc[:, j * P:(j + 1) * P],
                                         rhs=wcat_s[:], start=True, stop=True)
                        dstg = stage[:, j * 256:(j + 1) * 256]
                        if j % 2 == 0:
                            nc.scalar.copy(out=dstg, in_=ps[:])
                        else:
                            nc.vector.tensor_copy(out=dstg, in_=ps[:])
                    st3 = stage[:].rearrange("p (j c) -> p j c", c=256)
                    for d in range(2):
                        for t in range(NSH):
                            dv = T[d][t * RSHT:t * RSHT + RSH, :].rearrange(
                                "(q n) c -> q n c", n=NB)[:, g0:g0 + nb, :]
                            nc.sync.dma_start(
                                out=dv,
                                in_=st3[32 * t:32 * (t + 1), 0:nb,
                                        d * 128:(d + 1) * 128])

                # ---------------- node phase: aK (own blocks) ----------------
                NB2 = 7
                for b0 in range(0, NBLK_C, NB2):
                    nb = min(NB2, NBLK_C - b0)
                    xo = xpool.tile([P, NB2 * P], fp16, tag="xo")
                    nc.sync.dma_start(out=xo[:, 0:nb * P],
                                      in_=p_xo[:, b0 * P:(b0 + nb) * P])
                    stga = spool.tile([P, NB2 * 16], fp16, tag="stga")
                    for j in range(nb):
                        psa = npsum.tile([P, 16], f32, tag="npsa")
                        nc.tensor.matmul(out=psa[:], lhsT=xo[:, j * P:(j + 1) * P],
                                         rhs=wdst_s[:], start=True, stop=True)
                        if j % 2 == 0:
                            nc.scalar.copy(out=stga[:, j * 16:(j + 1) * 16], in_=psa[:])
                        else:
                            nc.vector.tensor_copy(out=stga[:, j * 16:(j + 1) * 16],
                                                  in_=psa[:])
                    av = aK[b0 * P:(b0 + nb) * P, 0:16].rearrange(
                        "(j p) c -> p j c", p=P)
                    nc.sync.dma_start(
                        out=av, in_=stga[:].rearrange("p (j c) -> p j c", c=16)[:, 0:nb, :])

            # ---------------- edge phase ----------------
            if build_stage == 1:
                for b in range(NBLK_C):
                    ofin = dpool.tile([P, 128], f32, tag="ofin")
                    nc.vector.tensor_copy(out=ofin[:], in_=bias_s[:])
                    nc.sync.dma_start(out=p_out[b * P:(b + 1) * P, :], in_=ofin[:])
            for g in (range(NG) if build_stage >= 2 or build_stage in (20, 21, 22) else []):
                ods = {}
                for d in range(2):
                    Sg = int(SG[d, g])
                    gTcols = Sg * 8
                    idxt = ipool.tile([P, SGMAX * 8], i16, tag="idxt")
                    nc.sync.dma_start(
                        out=idxt[:, 0:gTcols],
                        in_=p_idxT[:, int(itofs[d, g, 0]):int(itofs[d, g, 0]) + gTcols])
                    idxk = ipool.tile([P, SGMAX * 8], i16, tag="idxk")
                    nc.sync.dma_start(
                        out=idxk[:, 0:gTcols],
                        in_=p_idxK[:, int(ikofs[d, g]):int(ikofs[d, g]) + gTcols])

                    gh = gpool.tile([P, SGMAX * P], fp16, tag="gh")
                    gh3 = gh[:].rearrange("p (s c) -> p s c", c=P)
                    if build_stage == 21:
                        nc.vector.tensor_scalar(
                            out=gh[:], in0=iota_s[:, 0:1].to_broadcast([P, SGMAX * P]),
                            scalar1=0.0, scalar2=None, op0=mybir.AluOpType.mult)
                    for t in (range(NSH) if build_stage != 21 else []):
                        sl = int(seglen[d, g, t])
                        cb = int(colbase[d, g, t])
                        io = int(itofs[d, g, t] - itofs[d, g, 0])
                        base = 0 if build_stage == 22 else t * RSHT
                        nc.gpsimd.dma_gather(
                            out_ap=gh3[:, cb:cb + sl // 128, :],
                            in_ap=T[d][base:base + RSHT, :],
                            idxs_ap=idxt[:, io:io + sl // 16],
                            num_idxs=sl, num_idxs_reg=sl, elem_size=P,
                            single_packet=False, queue_num=t % 4)
                    agh = apool.tile([P, SGMAX * P], fp16, tag="agh")
                    agh3 = agh[:].rearrange("p (s c) -> p s c", c=P)
                    if build_stage != 20:
                        # split across all 4 SWDGE queues to balance per-queue
                        # descriptor load with the four T-gathers
                        ck = (Sg + 3) // 4
                        cb2 = 0
                        for ci in range(4):
                            cw = min(ck, Sg - cb2)
                            if cw <= 0:
                                break
                            nc.gpsimd.dma_gather(
                                out_ap=agh3[:, cb2:cb2 + cw, :],
                                in_ap=aK[0:NBLK_C * P + 1, :],
                                idxs_ap=idxk[:, cb2 * 8:(cb2 + cw) * 8],
                                num_idxs=cw * 128, num_idxs_reg=cw * 128,
                                elem_size=P, single_packet=False,
                                queue_num=(d + ci) % 4)
                            cb2 += cw
                    else:
                        nc.vector.tensor_scalar(out=agh[:], in0=gh[:],
                                                scalar1=0.0, scalar2=None,
                                                op0=mybir.AluOpType.mult)

                    if build_stage in (2, 20, 21, 22):
                        od2 = odpool.tile([P, 128], f32, tag=f"od{d}_0s2")
                        nc.vector.tensor_copy(out=od2[:], in_=gh[:, 0:128])
                        od2b = odpool.tile([P, 128], f32, tag=f"od{d}_1s2")
                        nc.vector.tensor_copy(out=od2b[:], in_=agh[:, 0:128])
                        for q in range(G):
                            ods[(d, q)] = od2 if q % 2 == 0 else od2b
                        continue
                    # alpha_src = <h, a_src_d> per head
                    tmp = tpool.tile([P, SGMAX * P], fp16, tag="tmp")
                    nc.vector.tensor_tensor(
                        out=tmp[:, 0:Sg * P].rearrange("p (s h c) -> p s h c",
                                                       h=HEADS, c=C),
                        in0=gh[:, 0:Sg * P].rearrange("p (s h c) -> p s h c",
                                                      h=HEADS, c=C),
                        in1=asrc_s[:, d * 128:(d + 1) * 128].rearrange(
                            "p (o h c) -> p o h c", o=1, h=HEADS, c=C
                        ).to_broadcast([P, Sg, HEADS, C]),
                        op=mybir.AluOpType.mult)
                    asr = smpool.tile([P, SGMAX * 8], f32, tag="asr")
                    nc.vector.tensor_reduce(
                        out=asr[:, 0:Sg * 8],
                        in_=tmp[:, 0:Sg * P].rearrange("p (sh c) -> p sh c", c=C),
                        axis=mybir.AxisListType.X, op=mybir.AluOpType.add)

                    aex = smpool.tile([P, SGMAX * 8], fp16, tag="aex")
                    nc.vector.tensor_tensor(
                        out=aex[:, 0:Sg * 8].rearrange("p (s h) -> p s h", h=8),
                        in0=asr[:, 0:Sg * 8].rearrange("p (s h) -> p s h", h=8),
                        in1=agh3[:, 0:Sg, d * 8:(d + 1) * 8],
                        op=mybir.AluOpType.add)
                    lrl0 = smpool.tile([P, SGMAX * 8], fp16, tag="lrl0")
                    nc.vector.tensor_scalar(out=lrl0[:, 0:Sg * 8],
                                            in0=aex[:, 0:Sg * 8],
                                            scalar1=NEG_SLOPE, scalar2=None,
                                            op0=mybir.AluOpType.mult)
                    lrl = smpool.tile([P, SGMAX * 8], fp16, tag="lrl")
                    nc.vector.tensor_tensor(out=lrl[:, 0:Sg * 8],
                                            in0=aex[:, 0:Sg * 8],
                                            in1=lrl0[:, 0:Sg * 8],
                                            op=mybir.AluOpType.max)
                    ex = smpool.tile([P, SGMAX * 8], fp16, tag="ex")
                    nc.scalar.activation(out=ex[:, 0:Sg * 8], in_=lrl[:, 0:Sg * 8],
                                         func=mybir.ActivationFunctionType.Exp)

                    # msg slots: [ex*h (128) | ex (8)]
                    msg = mpool.tile([P, SGMAX * 136], fp16, tag="msg")
                    msg3 = msg[:].rearrange("p (s c) -> p s c", c=136)
                    nc.vector.tensor_tensor(
                        out=msg3[:, 0:Sg, 0:128].rearrange("p s (h c) -> p s h c",
                                                           c=C),
                        in0=gh3[:, 0:Sg, :].rearrange("p s (h c) -> p s h c", c=C),
                        in1=ex[:, 0:Sg * 8].rearrange("p (s h o) -> p s h o",
                                                      h=8, o=1
                                                      ).to_broadcast([P, Sg, 8, C]),
                        op=mybir.AluOpType.mult)
                    nc.vector.tensor_copy(
                        out=msg3[:, 0:Sg, 128:136],
                        in_=ex[:, 0:Sg * 8].rearrange("p (s h) -> p s h", h=8))

                    if build_stage == 3:
                        for q in range(G):
                            od3 = odpool.tile([P, 128], f32, tag=f"od{d}_{q}")
                            nc.vector.tensor_copy(out=od3[:], in_=msg[:, 0:128])
                            ods[(d, q)] = od3
                        continue
                    for q in range(G):
                        ncq = int(ncolsa[d, g, q].sum())
                        lo = int(lblofs[d, g, q])
                        oneh = opool.tile([P, NCMAX * P], fp16, tag="oneh")
                        nc.vector.tensor_tensor(
                            out=oneh[:, 0:ncq * P].rearrange("p (n e) -> p n e",
                                                             e=P),
                            in0=lbl_s[:, lo:lo + ncq].rearrange(
                                "p (n o) -> p n o", o=1).to_broadcast([P, ncq, P]),
                            in1=iota_s[:, 0:ncq * P].rearrange("p (n e) -> p n e",
                                                               e=P),
                            op=mybir.AluOpType.is_equal)
                        pd = ppool.tile([P, 136], f32, tag="pd")
                        i = 0
                        for t in range(NSH):
                            for w in range(int(ncolsa[d, g, q, t])):
                                cc = int(c0a[d, g, q, t]) + w
                                nc.tensor.matmul(
                                    out=pd[:],
                                    lhsT=oneh[:, i * P:(i + 1) * P],
                                    rhs=msg[:, cc * 136:(cc + 1) * 136],
                                    start=(i == 0), stop=(i == ncq - 1))
                                i += 1
                        den = dpool.tile([P, 8], f32, tag="den")
                        nc.vector.tensor_scalar(out=den[:], in0=pd[:, 128:136],
                                                scalar1=1e-30, scalar2=None,
                                                op0=mybir.AluOpType.add)
                        rec = dpool.tile([P, 8], f32, tag="rec")
                        nc.vector.reciprocal(out=rec[:], in_=den[:])
                        od = odpool.tile([P, 128], f32, tag=f"od{d}_{q}")
                        nc.vector.tensor_tensor(
                            out=od[:].rearrange("p (h c) -> p h c", c=C),
                            in0=pd[:, 0:128].rearrange("p (h c) -> p h c", c=C),
                            in1=rec[:].rearrange("p (h o) -> p h o", o=1
                                                 ).to_broadcast([P, 8, C]),
                            op=mybir.AluOpType.mult)
                        ods[(d, q)] = od

                for q in range(G):
                    osum = dpool.tile([P, 128], f32, tag="osum")
                    nc.vector.tensor_tensor(out=osum[:], in0=ods[(0, q)][:],
                                            in1=ods[(1, q)][:],
                                            op=mybir.AluOpType.add)
                    ofin = dpool.tile([P, 128], f32, tag="ofin")
                    nc.vector.tensor_tensor(out=ofin[:], in0=osum[:], in1=bias_s[:],
                                            op=mybir.AluOpType.add)
                    b = g * G + q
                    nc.sync.dma_start(out=p_out[b * P:(b + 1) * P, :], in_=ofin[:])

    nc.compile()
    return nc


def _host_inputs(meta, x, W_in, a_src_in, a_dst_in, b_in, W_out, a_src_out,
                 a_dst_out, b_out, idxT, idxK, lbl, n_nodes, n_cores):
    NB, NPAD, NBLK_C, NCMAX = meta["NB"], meta["NPAD"], meta["NBLK_C"], meta["NCMAX"]
    Vdst_in = np.stack([W_in[:, h * C:(h + 1) * C] @ a_dst_in[h]
                        for h in range(HEADS)], 1)   # [D_IN, HEADS]
    Vdst_out = np.stack([W_out[:, h * C:(h + 1) * C] @ a_dst_out[h]
                         for h in range(HEADS)], 1)
    wcat = np.concatenate([W_in, W_out], axis=1).astype(np.float16)
    wdst = np.concatenate([Vdst_in, Vdst_out], axis=1).astype(np.float16)
    asrcc = np.tile(np.concatenate([a_src_in.reshape(-1), a_src_out.reshape(-1)]
                                   ).astype(np.float16)[None, :], (P, 1))
    xT = np.zeros((D_IN, NPAD), np.float16)
    xT[:, :n_nodes] = x.T.astype(np.float16)
    iota = np.tile(np.arange(P, dtype=np.float16), (P, NCMAX)).reshape(P, NCMAX * P)
    bias = np.tile((b_in + b_out).astype(np.float32)[None, :], (P, 1))
    zrow = np.zeros((1, 128), np.float16)
    nrow = np.full((1, 128), -30000.0, np.float16)

    shared = dict(xT=xT, wcat=wcat, wdst=wdst, asrcc=asrcc, iota=iota,
                  bias=bias, zrow=zrow, nrow=nrow)
    in_maps = []
    for k in range(n_cores):
        xo = xT[:, k * NBLK_C * P:(k + 1) * NBLK_C * P]
        in_maps.append(dict(shared, xo=np.ascontiguousarray(xo),
                            idxT=idxT[k], idxK=idxK[k], lbl=lbl[k]))
    return in_maps


def kernel(x, ei, W_in, a_src_in, a_dst_in, b_in, W_out, a_src_out, a_dst_out,
           b_out, n_cores=8, G=7):
    from concourse.bass_utils import run_bass_kernel_spmd

    x = np.asarray(x, np.float32)
    ei = np.asarray(ei, np.int32)
    n_nodes = x.shape[0]
    meta, idxT, idxK, lbl = _prep(ei, n_nodes, n_cores, G)
    nc = _build(meta, n_cores)
    in_maps = _host_inputs(meta, x,
                           np.asarray(W_in, np.float32),
                           np.asarray(a_src_in, np.float32),
                           np.asarray(a_dst_in, np.float32),
                           np.asarray(b_in, np.float32),
                           np.asarray(W_out, np.float32),
                           np.asarray(a_src_out, np.float32),
                           np.asarray(a_dst_out, np.float32),
                           np.asarray(b_out, np.float32),
                           idxT, idxK, lbl, n_nodes, n_cores)
    _LAST.update(nc=nc, in_maps=in_maps, n_cores=n_cores, meta=meta)
    res = run_bass_kernel_spmd(nc, in_maps, list(range(n_cores)))
    full = np.concatenate([res.results[k]["out"] for k in range(n_cores)], axis=0)
    return full[:n_nodes].astype(np.float32)


# revision 4
# speedup vs baseline: 11.6192x; 1.1002x over previous
"""GAT DirSeq Trainium2 kernel, v3.

Key difference vs v2 (baseline): the edge phase uses a few large
`dma_gather` instructions (int16-indexed, 256B rows, 4-way sharded tables)
instead of thousands of small `indirect_dma_start` calls. SWDGE has ~1us
fixed cost per instruction and ~0.34ns per descriptor, so instruction count
is everything.

Layout:
  - Nodes scrambled: node r -> k = (r%128)*NB + r//128; shard t = (r%128)//32,
    within-shard row16 = ((r%128)%32)*NB + r//128 (< 32768 -> int16 ok).
  - T0/T1 tables [4*(RSH+1), 128] fp16: h_in / h_out rows (256B, last row of
    each shard = zero sentinel). Node-phase stores are contiguous per
    partition-group.
  - alpha_src is recomputed per edge from the gathered h row via an on-chip
    dot with a_src (mult + reduce), so it needs no table.
  - alpha_dst comes from a per-core compact table aK [NBLK_C*128+1, 128]
    (cols 0:16 = [adst_in|adst_out]) indexed by block-local key id, gathered
    with one dma_gather per group; filled by a mini node-phase over the
    core's own x slice (per-core x_own parameter keeps the program SPMD).
  - Edge phase: destination blocks grouped G at a time; per (group, dir):
    4 h-gathers (one per source shard) + 1 aK-gather; softmax as
    unnormalized weighted sums; scatter via one-hot matmuls with per-block
    masked labels (shared boundary columns are masked by label 999).
"""

import math
import numpy as np

N = 100000
E = 800000
D_IN = 128
HEADS = 8
C = 16
NEG_SLOPE = 0.2
P = 128
NSH = 4
_LAST = {}


# ------------------------------------------------------------------ host prep
def _prep(ei, n_nodes, n_cores, G):
    """Build per-core gather index streams, label arrays and static metadata."""
    NBLK_C = int(math.ceil(n_nodes / P / n_cores))
    NB = NBLK_C * n_cores
    NPAD = NB * P
    RSH = 32 * NB              # rows per shard (int16-addressable)
    NG = NBLK_C // G           # groups per core
    assert NBLK_C % G == 0 and NB % NSH == 0 and RSH < 32768

    AKS = NBLK_C * P           # aK sentinel row (per-core compact table)
    meta = {"NBLK_C": NBLK_C, "NB": NB, "NPAD": NPAD, "RSH": RSH, "NG": NG,
            "G": G, "AKS": AKS}

    src, dst = ei[0].astype(np.int64), ei[1].astype(np.int64)
    dirs = []
    for d in range(2):
        key = dst if d == 0 else src   # grouping (destination) node
        oth = src if d == 0 else dst   # message source node
        kb = key >> 7
        core = kb // NBLK_C
        qb = kb % NBLK_C
        g = qb // G
        q = qb % G
        klm = key & 127
        t = (oth & 127) >> 5
        row16 = ((oth & 127) & 31) * NB + (oth >> 7)
        kloc = qb * P + klm
        seg = ((core * NG + g) * NSH + t)
        order = np.lexsort((qb, seg))
        dirs.append(dict(core=core[order], g=g[order], q=q[order],
                         klm=klm[order], t=t[order], row16=row16[order],
                         kloc=kloc[order], seg=seg[order], qb=qb[order]))

    # static segment lengths (max over cores, padded to 128)
    seglen = np.zeros((2, NG, NSH), np.int64)
    for d in range(2):
        cnt = np.bincount(dirs[d]["seg"] % (NG * NSH) +
                          dirs[d]["core"] * (NG * NSH),
                          minlength=n_cores * NG * NSH
                          ).reshape(n_cores, NG, NSH)
        m = cnt.max(axis=0)
        seglen[d] = np.maximum(128, ((m + 127) // 128) * 128)

    colbase = np.zeros((2, NG, NSH), np.int64)   # column offset inside group
    SG = np.zeros((2, NG), np.int64)
    for d in range(2):
        for g in range(NG):
            cb = 0
            for t in range(NSH):
                colbase[d, g, t] = cb
                cb += seglen[d, g, t] // 128
            SG[d, g] = cb
    SGMAX = int(SG.max())

    # per-edge slot position within its group's stream
    for d in range(2):
        dd = dirs[d]
        segid = dd["core"] * (NG * NSH) + dd["seg"] % (NG * NSH)
        # rank within segment
        starts = np.searchsorted(segid, np.arange(n_cores * NG * NSH))
        rank = np.arange(segid.size) - starts[segid]
        dd["slot"] = colbase[d, dd["g"], dd["t"]] * 128 + rank
        dd["col"] = dd["slot"] >> 7

    # per-(d,g,q,t) column ranges (union over cores) and label offsets.
    # A block's edges form one contiguous run per source shard segment.
    c0 = np.zeros((2, NG, G, NSH), np.int64)
    nc_ = np.zeros((2, NG, G, NSH), np.int64)
    for d in range(2):
        dd = dirs[d]
        gqt = (dd["g"] * G + dd["q"]) * NSH + dd["t"]
        cmin = np.full(NG * G * NSH, 1 << 30, np.int64)
        cmax = np.full(NG * G * NSH, -1, np.int64)
        np.minimum.at(cmin, gqt, dd["col"])
        np.maximum.at(cmax, gqt, dd["col"])
        has = cmax >= 0
        c0[d][has.reshape(NG, G, NSH)] = cmin[has]
        nc_[d].reshape(-1)[has] = (cmax - cmin + 1)[has]
    # blocks with zero edges anywhere: give one inert column
    tot = nc_.sum(axis=3)
    for d in range(2):
        for g in range(NG):
            for q in range(G):
                if tot[d, g, q] == 0:
                    nc_[d, g, q, 0] = 1
    NCMAX = int(nc_.sum(axis=3).max())
    lblofs = np.zeros((2, NG, G), np.int64)       # label col offset per block
    tofsw = np.zeros((2, NG, G, NSH), np.int64)   # within-block per-shard ofs
    acc = 0
    for d in range(2):
        for g in range(NG):
            for q in range(G):
                lblofs[d, g, q] = acc
                w = 0
                for t in range(NSH):
                    tofsw[d, g, q, t] = w
                    w += nc_[d, g, q, t]
                acc += w
    LCOLS = int(acc)

    # group stream offsets (common to all cores)
    gofs = np.zeros((2, NG), np.int64)        # in slots
    itofs = np.zeros((2, NG, NSH), np.int64)  # idxT col offsets
    ikofs = np.zeros((2, NG), np.int64)       # idxK col offsets
    L = np.zeros(2, np.int64)
    ti, ki = 0, 0
    for d in range(2):
        o = 0
        for g in range(NG):
            gofs[d, g] = o
            for t in range(NSH):
                itofs[d, g, t] = ti
                ti += seglen[d, g, t] // 16
            ikofs[d, g] = ki
            ki += (SG[d, g] * 128) // 16
            o += SG[d, g] * 128
        L[d] = o
    TCOLS, KCOLS = int(ti), int(ki)

    # per-core streams
    idxT = np.full((n_cores, TCOLS * 16), RSH, np.int16)
    idxK = np.full((n_cores, KCOLS * 16), AKS, np.int16)
    lbl = np.full((n_cores, P, LCOLS), 999.0, np.float16)
    for d in range(2):
        dd = dirs[d]
        # stream position of each edge inside its core's direction stream
        spos = gofs[d, dd["g"]] + dd["slot"]
        # idxT: wrapped per segment; compute wrapped linear position:
        # within segment (g,t): j = slot - colbase*128 ... wrapped [16, len/16]
        j = dd["slot"] - colbase[d, dd["g"], dd["t"]] * 128
        wrap = (itofs[d, dd["g"], dd["t"]] + j // 16) * 16 + (j % 16)
        jk = dd["slot"]
        wrapk = (ikofs[d, dd["g"]] + jk // 16) * 16 + (jk % 16)
        pcol = (lblofs[d, dd["g"], dd["q"]] +
                tofsw[d, dd["g"], dd["q"], dd["t"]] +
                dd["col"] - c0[d, dd["g"], dd["q"], dd["t"]])
        for k in range(n_cores):
            m = dd["core"] == k
            idxT[k][wrap[m]] = dd["row16"][m]
            idxK[k][wrapk[m]] = dd["kloc"][m]
            lbl[k][dd["slot"][m] & 127, pcol[m]] = dd["klm"][m].astype(np.float16)
    # [16, cols] wrap -> replicate to 128 partitions
    idxT = np.ascontiguousarray(
        np.tile(idxT.reshape(n_cores, TCOLS, 16).transpose(0, 2, 1), (1, 8, 1)))
    idxK = np.ascontiguousarray(
        np.tile(idxK.reshape(n_cores, KCOLS, 16).transpose(0, 2, 1), (1, 8, 1)))

    meta.update(seglen=seglen, colbase=colbase, SG=SG, SGMAX=SGMAX, gofs=gofs,
                itofs=itofs, ikofs=ikofs, c0=c0, ncols=nc_, NCMAX=NCMAX,
                lblofs=lblofs, tofsw=tofsw, LCOLS=LCOLS, TCOLS=TCOLS,
                KCOLS=KCOLS)
    return meta, idxT, idxK, lbl


def _build(meta, n_cores, compile_only=True, build_stage=4):
    import concourse.bacc as bacc
    import concourse.mybir as mybir
    import concourse.tile as tile

    fp16 = mybir.dt.float16
    f32 = mybir.dt.float32
    i16 = mybir.dt.int16

    NBLK_C, NB, NPAD, RSH = meta["NBLK_C"], meta["NB"], meta["NPAD"], meta["RSH"]
    NG, G, AKS = meta["NG"], meta["G"], meta["AKS"]
    SGMAX, NCMAX, LCOLS = meta["SGMAX"], meta["NCMAX"], meta["LCOLS"]
    TCOLS, KCOLS = meta["TCOLS"], meta["KCOLS"]
    seglen, colbase, SG = meta["seglen"], meta["colbase"], meta["SG"]
    gofs, itofs, ikofs = meta["gofs"], meta["itofs"], meta["ikofs"]
    c0a, ncolsa, lblofs = meta["c0"], meta["ncols"], meta["lblofs"]
    RSHT = RSH + 1

    nc = bacc.Bacc(None, target_bir_lowering=False, debug=False,
                   num_swdge_queues=4)

    p_xT = nc.declare_dram_parameter("xT", [P, NPAD], fp16, isOutput=False)
    p_xo = nc.declare_dram_parameter("xo", [P, NBLK_C * P], fp16, isOutput=False)
    p_wcat = nc.declare_dram_parameter("wcat", [P, 256], fp16, isOutput=False)
    p_wdst = nc.declare_dram_parameter("wdst", [P, 16], fp16, isOutput=False)
    p_asrc = nc.declare_dram_parameter("asrcc", [P, 256], fp16, isOutput=False)
    p_iota = nc.declare_dram_parameter("iota", [P, NCMAX * P], fp16, isOutput=False)
    p_bias = nc.declare_dram_parameter("bias", [P, 128], f32, isOutput=False)
    p_zrow = nc.declare_dram_parameter("zrow", [1, 128], fp16, isOutput=False)
    p_nrow = nc.declare_dram_parameter("nrow", [1, 128], fp16, isOutput=False)
    p_idxT = nc.declare_dram_parameter("idxT", [P, TCOLS], i16, isOutput=False)
    p_idxK = nc.declare_dram_parameter("idxK", [P, KCOLS], i16, isOutput=False)
    p_lbl = nc.declare_dram_parameter("lbl", [P, LCOLS], fp16, isOutput=False)
    p_out = nc.declare_dram_parameter("out", [NBLK_C * P, 128], f32, isOutput=True)

    T = [nc.dram_tensor("T0", [NSH * RSHT, 128], fp16),
         nc.dram_tensor("T1", [NSH * RSHT, 128], fp16)]
    aK = nc.dram_tensor("aK", [NBLK_C * P + 1, 128], fp16)

    NBAT = 8

    with tile.TileContext(nc) as tc:
        with (
            tc.tile_pool(name="const", bufs=1) as cpool,
            tc.tile_pool(name="gh", bufs=2) as gpool,
            tc.tile_pool(name="agh", bufs=2) as apool,
            tc.tile_pool(name="tmp", bufs=1) as tpool,
            tc.tile_pool(name="msg", bufs=2) as mpool,
            tc.tile_pool(name="sm", bufs=2) as smpool,
            tc.tile_pool(name="oneh", bufs=3) as opool,
            tc.tile_pool(name="idx", bufs=2) as ipool,
            tc.tile_pool(name="epi", bufs=2) as dpool,
            tc.tile_pool(name="odp", bufs=2) as odpool,
            tc.tile_pool(name="pd", bufs=4, space="PSUM") as ppool,
        ):
            wcat_s = cpool.tile([P, 256], fp16)
            nc.sync.dma_start(out=wcat_s[:], in_=p_wcat[:])
            wdst_s = cpool.tile([P, 16], fp16)
            nc.sync.dma_start(out=wdst_s[:], in_=p_wdst[:])
            asrc_s = cpool.tile([P, 256], fp16)
            nc.sync.dma_start(out=asrc_s[:], in_=p_asrc[:])
            iota_s = cpool.tile([P, NCMAX * P], fp16)
            nc.sync.dma_start(out=iota_s[:], in_=p_iota[:])
            bias_s = cpool.tile([P, 128], f32)
            nc.sync.dma_start(out=bias_s[:], in_=p_bias[:])
            lbl_s = cpool.tile([P, LCOLS], fp16)
            nc.sync.dma_start(out=lbl_s[:], in_=p_lbl[:])

            # sentinel rows
            for d in range(2):
                for t in range(NSH):
                    nc.sync.dma_start(out=T[d][t * RSHT + RSH:t * RSHT + RSHT, :],
                                      in_=p_zrow[:])
            nc.sync.dma_start(out=aK[NBLK_C * P:NBLK_C * P + 1, :], in_=p_nrow[:])

            # ---------------- node phase ----------------
            with (
                tc.tile_pool(name="xc", bufs=2) as xpool,
                tc.tile_pool(name="stage", bufs=2) as spool,
                tc.tile_pool(name="nps", bufs=2, space="PSUM") as npsum,
            ):
                # aK (own blocks) first: unblocks edge-phase aK gathers early
                NB2 = 7
                for b0 in range(0, NBLK_C, NB2):
                    nb = min(NB2, NBLK_C - b0)
                    xo = xpool.tile([P, NB2 * P], fp16, tag="xo")
                    nc.sync.dma_start(out=xo[:, 0:nb * P],
                                      in_=p_xo[:, b0 * P:(b0 + nb) * P])
                    stga = spool.tile([P, NB2 * 16], fp16, tag="stga")
                    for j in range(nb):
                        psa = npsum.tile([P, 16], f32, tag="npsa")
                        nc.tensor.matmul(out=psa[:], lhsT=xo[:, j * P:(j + 1) * P],
                                         rhs=wdst_s[:], start=True, stop=True)
                        if j % 2 == 0:
                            nc.scalar.copy(out=stga[:, j * 16:(j + 1) * 16], in_=psa[:])
                        else:
                            nc.vector.tensor_copy(out=stga[:, j * 16:(j + 1) * 16],
                                                  in_=psa[:])
                    av = aK[b0 * P:(b0 + nb) * P, 0:16].rearrange(
                        "(j p) c -> p j c", p=P)
                    nc.sync.dma_start(
                        out=av, in_=stga[:].rearrange("p (j c) -> p j c", c=16)[:, 0:nb, :])

                # h tables: one pass per direction so T0 completes before
                # T1 work, letting d=0 edge gathers overlap the T1 pass
                for dp in range(2):
                    for g0 in range(0, NB, NBAT):
                        nb = min(NBAT, NB - g0)
                        xc = xpool.tile([P, NBAT * P], fp16, tag="xc")
                        nc.sync.dma_start(out=xc[:, 0:nb * P],
                                          in_=p_xT[:, g0 * P:(g0 + nb) * P])
                        stage = spool.tile([P, NBAT * 128], fp16, tag="stage")
                        for j in range(nb):
                            ps = npsum.tile([P, 128], f32, tag="nps")
                            nc.tensor.matmul(out=ps[:],
                                             lhsT=xc[:, j * P:(j + 1) * P],
                                             rhs=wcat_s[:, dp * 128:(dp + 1) * 128],
                                             start=True, stop=True)
                            dstg = stage[:, j * 128:(j + 1) * 128]
                            if j % 2 == 0:
                                nc.scalar.copy(out=dstg, in_=ps[:])
                            else:
                                nc.vector.tensor_copy(out=dstg, in_=ps[:])
                        st3 = stage[:].rearrange("p (j c) -> p j c", c=128)
                        for t in range(NSH):
                            dv = T[dp][t * RSHT:t * RSHT + RSH, :].rearrange(
                                "(q n) c -> q n c", n=NB)[:, g0:g0 + nb, :]
                            nc.sync.dma_start(
                                out=dv, in_=st3[32 * t:32 * (t + 1), 0:nb, :])

            # ---------------- edge phase ----------------
            if build_stage == 1:
                for b in range(NBLK_C):
                    ofin = dpool.tile([P, 128], f32, tag="ofin")
                    nc.vector.tensor_copy(out=ofin[:], in_=bias_s[:])
                    nc.sync.dma_start(out=p_out[b * P:(b + 1) * P, :], in_=ofin[:])
            for g in (range(NG) if build_stage >= 2 or build_stage in (20, 21, 22) else []):
                ods = {}
                for d in range(2):
                    Sg = int(SG[d, g])
                    gTcols = Sg * 8
                    idxt = ipool.tile([P, SGMAX * 8], i16, tag="idxt")
                    nc.scalar.dma_start(
                        out=idxt[:, 0:gTcols],
                        in_=p_idxT[:, int(itofs[d, g, 0]):int(itofs[d, g, 0]) + gTcols])
                    idxk = ipool.tile([P, SGMAX * 8], i16, tag="idxk")
                    nc.scalar.dma_start(
                        out=idxk[:, 0:gTcols],
                        in_=p_idxK[:, int(ikofs[d, g]):int(ikofs[d, g]) + gTcols])

                    gh = gpool.tile([P, SGMAX * P], fp16, tag="gh")
                    gh3 = gh[:].rearrange("p (s c) -> p s c", c=P)
                    if build_stage == 21:
                        nc.vector.tensor_scalar(
                            out=gh[:], in0=iota_s[:, 0:1].to_broadcast([P, SGMAX * P]),
                            scalar1=0.0, scalar2=None, op0=mybir.AluOpType.mult)
                    for t in (range(NSH) if build_stage != 21 else []):
                        sl = int(seglen[d, g, t])
                        cb = int(colbase[d, g, t])
                        io = int(itofs[d, g, t] - itofs[d, g, 0])
                        base = 0 if build_stage == 22 else t * RSHT
                        nc.gpsimd.dma_gather(
                            out_ap=gh3[:, cb:cb + sl // 128, :],
                            in_ap=T[d][base:base + RSHT, :],
                            idxs_ap=idxt[:, io:io + sl // 16],
                            num_idxs=sl, num_idxs_reg=sl, elem_size=P,
                            single_packet=False, queue_num=t % 4)
                    agh = apool.tile([P, SGMAX * P], fp16, tag="agh")
                    agh3 = agh[:].rearrange("p (s c) -> p s c", c=P)
                    if build_stage != 20:
                        # split across all 4 SWDGE queues to balance per-queue
                        # descriptor load with the four T-gathers
                        ck = (Sg + 3) // 4
                        cb2 = 0
                        for ci in range(4):
                            cw = min(ck, Sg - cb2)
                            if cw <= 0:
                                break
                            nc.gpsimd.dma_gather(
                                out_ap=agh3[:, cb2:cb2 + cw, :],
                                in_ap=aK[0:NBLK_C * P + 1, :],
                                idxs_ap=idxk[:, cb2 * 8:(cb2 + cw) * 8],
                                num_idxs=cw * 128, num_idxs_reg=cw * 128,
                                elem_size=P, single_packet=False,
                                queue_num=(d + ci) % 4)
                            cb2 += cw
                    else:
                        nc.vector.tensor_scalar(out=agh[:], in0=gh[:],
                                                scalar1=0.0, scalar2=None,
                                                op0=mybir.AluOpType.mult)

                    if build_stage in (2, 20, 21, 22):
                        od2 = odpool.tile([P, 128], f32, tag=f"od{d}_0s2")
                        nc.vector.tensor_copy(out=od2[:], in_=gh[:, 0:128])
                        od2b = odpool.tile([P, 128], f32, tag=f"od{d}_1s2")
                        nc.vector.tensor_copy(out=od2b[:], in_=agh[:, 0:128])
                        for q in range(G):
                            ods[(d, q)] = od2 if q % 2 == 0 else od2b
                        continue
                    # alpha_src = <h, a_src_d> per head
                    tmp = tpool.tile([P, SGMAX * P], fp16, tag="tmp")
                    nc.vector.tensor_tensor(
                        out=tmp[:, 0:Sg * P].rearrange("p (s h c) -> p s h c",
                                                       h=HEADS, c=C),
                        in0=gh[:, 0:Sg * P].rearrange("p (s h c) -> p s h c",
                                                      h=HEADS, c=C),
                        in1=asrc_s[:, d * 128:(d + 1) * 128].rearrange(
                            "p (o h c) -> p o h c", o=1, h=HEADS, c=C
                        ).to_broadcast([P, Sg, HEADS, C]),
                        op=mybir.AluOpType.mult)
                    asr = smpool.tile([P, SGMAX * 8], f32, tag="asr")
                    nc.vector.tensor_reduce(
                        out=asr[:, 0:Sg * 8],
                        in_=tmp[:, 0:Sg * P].rearrange("p (sh c) -> p sh c", c=C),
                        axis=mybir.AxisListType.X, op=mybir.AluOpType.add)

                    aex = smpool.tile([P, SGMAX * 8], fp16, tag="aex")
                    nc.vector.tensor_tensor(
                        out=aex[:, 0:Sg * 8].rearrange("p (s h) -> p s h", h=8),
                        in0=asr[:, 0:Sg * 8].rearrange("p (s h) -> p s h", h=8),
                        in1=agh3[:, 0:Sg, d * 8:(d + 1) * 8],
                        op=mybir.AluOpType.add)
                    lrl0 = smpool.tile([P, SGMAX * 8], fp16, tag="lrl0")
                    nc.vector.tensor_scalar(out=lrl0[:, 0:Sg * 8],
                                            in0=aex[:, 0:Sg * 8],
                                            scalar1=NEG_SLOPE, scalar2=None,
                                            op0=mybir.AluOpType.mult)
                    lrl = smpool.tile([P, SGMAX * 8], fp16, tag="lrl")
                    nc.vector.tensor_tensor(out=lrl[:, 0:Sg * 8],
                                            in0=aex[:, 0:Sg * 8],
                                            in1=lrl0[:, 0:Sg * 8],
                                            op=mybir.AluOpType.max)
                    ex = smpool.tile([P, SGMAX * 8], fp16, tag="ex")
                    nc.scalar.activation(out=ex[:, 0:Sg * 8], in_=lrl[:, 0:Sg * 8],
                                         func=mybir.ActivationFunctionType.Exp)

                    # msg slots: [ex*h (128) | ex (8)]
                    msg = mpool.tile([P, SGMAX * 136], fp16, tag="msg")
                    msg3 = msg[:].rearrange("p (s c) -> p s c", c=136)
                    nc.vector.tensor_tensor(
                        out=msg3[:, 0:Sg, 0:128].rearrange("p s (h c) -> p s h c",
                                                           c=C),
                        in0=gh3[:, 0:Sg, :].rearrange("p s (h c) -> p s h c", c=C),
                        in1=ex[:, 0:Sg * 8].rearrange("p (s h o) -> p s h o",
                                                      h=8, o=1
                                                      ).to_broadcast([P, Sg, 8, C]),
                        op=mybir.AluOpType.mult)
                    nc.vector.tensor_copy(
                        out=msg3[:, 0:Sg, 128:136],
                        in_=ex[:, 0:Sg * 8].rearrange("p (s h) -> p s h", h=8))

                    if build_stage == 3:
                        for q in range(G):
                            od3 = odpool.tile([P, 128], f32, tag=f"od{d}_{q}")
                            nc.vector.tensor_copy(out=od3[:], in_=msg[:, 0:128])
                            ods[(d, q)] = od3
                        continue
                    for q in range(G):
                        ncq = int(ncolsa[d, g, q].sum())
                        lo = int(lblofs[d, g, q])
                        oneh = opool.tile([P, NCMAX * P], fp16, tag="oneh")
                        nc.vector.tensor_tensor(
                            out=oneh[:, 0:ncq * P].rearrange("p (n e) -> p n e",
                                                             e=P),
                            in0=lbl_s[:, lo:lo + ncq].rearrange(
                                "p (n o) -> p n o", o=1).to_broadcast([P, ncq, P]),
                            in1=iota_s[:, 0:ncq * P].rearrange("p (n e) -> p n e",
                                                               e=P),
                            op=mybir.AluOpType.is_equal)
                        pd = ppool.tile([P, 136], f32, tag="pd")
                        i = 0
                        for t in range(NSH):
                            for w in range(int(ncolsa[d, g, q, t])):
                                cc = int(c0a[d, g, q, t]) + w
                                nc.tensor.matmul(
                                    out=pd[:],
                                    lhsT=oneh[:, i * P:(i + 1) * P],
                                    rhs=msg[:, cc * 136:(cc + 1) * 136],
                                    start=(i == 0), stop=(i == ncq - 1))
                                i += 1
                        den = dpool.tile([P, 8], f32, tag="den")
                        nc.vector.tensor_scalar(out=den[:], in0=pd[:, 128:136],
                                                scalar1=1e-30, scalar2=None,
                                                op0=mybir.AluOpType.add)
                        rec = dpool.tile([P, 8], f32, tag="rec")
                        nc.vector.reciprocal(out=rec[:], in_=den[:])
                        od = odpool.tile([P, 128], f32, tag=f"od{d}_{q}")
                        nc.vector.tensor_tensor(
                            out=od[:].rearrange("p (h c) -> p h c", c=C),
                            in0=pd[:, 0:128].rearrange("p (h c) -> p h c", c=C),
                            in1=rec[:].rearrange("p (h o) -> p h o", o=1
                                                 ).to_broadcast([P, 8, C]),
                            op=mybir.AluOpType.mult)
                        ods[(d, q)] = od

                for q in range(G):
                    osum = dpool.tile([P, 128], f32, tag="osum")
                    nc.vector.tensor_tensor(out=osum[:], in0=ods[(0, q)][:],
                                            in1=ods[(1, q)][:],
                                            op=mybir.AluOpType.add)
                    ofin = dpool.tile([P, 128], f32, tag="ofin")
                    nc.vector.tensor_tensor(out=ofin[:], in0=osum[:], in1=bias_s[:],
                                            op=mybir.AluOpType.add)
                    b = g * G + q
                    nc.sync.dma_start(out=p_out[b * P:(b + 1) * P, :], in_=ofin[:])

    nc.compile()
    return nc


def _host_inputs(meta, x, W_in, a_src_in, a_dst_in, b_in, W_out, a_src_out,
                 a_dst_out, b_out, idxT, idxK, lbl, n_nodes, n_cores):
    NB, NPAD, NBLK_C, NCMAX = meta["NB"], meta["NPAD"], meta["NBLK_C"], meta["NCMAX"]
    Vdst_in = np.stack([W_in[:, h * C:(h + 1) * C] @ a_dst_in[h]
                        for h in range(HEADS)], 1)   # [D_IN, HEADS]
    Vdst_out = np.stack([W_out[:, h * C:(h + 1) * C] @ a_dst_out[h]
                         for h in range(HEADS)], 1)
    wcat = np.concatenate([W_in, W_out], axis=1).astype(np.float16)
    wdst = np.concatenate([Vdst_in, Vdst_out], axis=1).astype(np.float16)
    asrcc = np.tile(np.concatenate([a_src_in.reshape(-1), a_src_out.reshape(-1)]
                                   ).astype(np.float16)[None, :], (P, 1))
    xT = np.zeros((D_IN, NPAD), np.float16)
    xT[:, :n_nodes] = x.T.astype(np.float16)
    iota = np.tile(np.arange(P, dtype=np.float16), (P, NCMAX)).reshape(P, NCMAX * P)
    bias = np.tile((b_in + b_out).astype(np.float32)[None, :], (P, 1))
    zrow = np.zeros((1, 128), np.float16)
    nrow = np.full((1, 128), -30000.0, np.float16)

    shared = dict(xT=xT, wcat=wcat, wdst=wdst, asrcc=asrcc, iota=iota,
                  bias=bias, zrow=zrow, nrow=nrow)
    in_maps = []
    for k in range(n_cores):
        xo = xT[:, k * NBLK_C * P:(k + 1) * NBLK_C * P]
        in_maps.append(dict(shared, xo=np.ascontiguousarray(xo),
                            idxT=idxT[k], idxK=idxK[k], lbl=lbl[k]))
    return in_maps


def kernel(x, ei, W_in, a_src_in, a_dst_in, b_in, W_out, a_src_out, a_dst_out,
           b_out, n_cores=8, G=7):
    from concourse.bass_utils import run_bass_kernel_spmd

    x = np.asarray(x, np.float32)
    ei = np.asarray(ei, np.int32)
    n_nodes = x.shape[0]
    meta, idxT, idxK, lbl = _prep(ei, n_nodes, n_cores, G)
    nc = _build(meta, n_cores)
    in_maps = _host_inputs(meta, x,
                           np.asarray(W_in, np.float32),
                           np.asarray(a_src_in, np.float32),
                           np.asarray(a_dst_in, np.float32),
                           np.asarray(b_in, np.float32),
                           np.asarray(W_out, np.float32),
                           np.asarray(a_src_out, np.float32),
                           np.asarray(a_dst_out, np.float32),
                           np.asarray(b_out, np.float32),
                           idxT, idxK, lbl, n_nodes, n_cores)
    _LAST.update(nc=nc, in_maps=in_maps, n_cores=n_cores, meta=meta)
    res = run_bass_kernel_spmd(nc, in_maps, list(range(n_cores)))
    full = np.concatenate([res.results[k]["out"] for k in range(n_cores)], axis=0)
    return full[:n_nodes].astype(np.float32)


# revision 5
# speedup vs baseline: 13.6694x; 1.1764x over previous
"""GAT DirSeq Trainium2 kernel, v3.

Key difference vs v2 (baseline): the edge phase uses a few large
`dma_gather` instructions (int16-indexed, 256B rows, 4-way sharded tables)
instead of thousands of small `indirect_dma_start` calls. SWDGE has ~1us
fixed cost per instruction and ~0.34ns per descriptor, so instruction count
is everything.

Layout:
  - Nodes scrambled: node r -> k = (r%128)*NB + r//128; shard t = (r%128)//32,
    within-shard row16 = ((r%128)%32)*NB + r//128 (< 32768 -> int16 ok).
  - T0/T1 tables [4*(RSH+1), 128] fp16: h_in / h_out rows (256B, last row of
    each shard = zero sentinel). Node-phase stores are contiguous per
    partition-group.
  - alpha_src is recomputed per edge from the gathered h row via an on-chip
    dot with a_src (mult + reduce), so it needs no table.
  - alpha_dst comes from a per-core compact table aK [NBLK_C*128+1, 128]
    (cols 0:16 = [adst_in|adst_out]) indexed by block-local key id, gathered
    with one dma_gather per group; filled by a mini node-phase over the
    core's own x slice (per-core x_own parameter keeps the program SPMD).
  - Edge phase: destination blocks grouped G at a time; per (group, dir):
    4 h-gathers (one per source shard) + 1 aK-gather; softmax as
    unnormalized weighted sums; scatter via one-hot matmuls with per-block
    masked labels (shared boundary columns are masked by label 999).
"""

import math
import numpy as np

N = 100000
E = 800000
D_IN = 128
HEADS = 8
C = 16
NEG_SLOPE = 0.2
P = 128
NSH = 4
_LAST = {}


# ------------------------------------------------------------------ host prep
def _prep(ei, n_nodes, n_cores, G):
    """Build per-core gather index streams, label arrays and static metadata."""
    NBLK_C = int(math.ceil(n_nodes / P / n_cores))
    NB = NBLK_C * n_cores
    NPAD = NB * P
    RSH = 32 * NB              # rows per shard (int16-addressable)
    NG = NBLK_C // G           # groups per core
    assert NBLK_C % G == 0 and NB % NSH == 0 and RSH < 32768

    AKS = NBLK_C * P           # aK sentinel row (per-core compact table)
    meta = {"NBLK_C": NBLK_C, "NB": NB, "NPAD": NPAD, "RSH": RSH, "NG": NG,
            "G": G, "AKS": AKS}

    src, dst = ei[0].astype(np.int64), ei[1].astype(np.int64)
    dirs = []
    for d in range(2):
        key = dst if d == 0 else src   # grouping (destination) node
        oth = src if d == 0 else dst   # message source node
        kb = key >> 7
        core = kb // NBLK_C
        qb = kb % NBLK_C
        g = qb // G
        q = qb % G
        klm = key & 127
        t = (oth & 127) >> 5
        row16 = ((oth & 127) & 31) * NB + (oth >> 7)
        kloc = qb * P + klm
        seg = ((core * NG + g) * NSH + t)
        order = np.lexsort((qb, seg))
        dirs.append(dict(core=core[order], g=g[order], q=q[order],
                         klm=klm[order], t=t[order], row16=row16[order],
                         kloc=kloc[order], seg=seg[order], qb=qb[order]))

    # static segment lengths (max over cores, padded to 128)
    seglen = np.zeros((2, NG, NSH), np.int64)
    for d in range(2):
        cnt = np.bincount(dirs[d]["seg"] % (NG * NSH) +
                          dirs[d]["core"] * (NG * NSH),
                          minlength=n_cores * NG * NSH
                          ).reshape(n_cores, NG, NSH)
        m = cnt.max(axis=0)
        seglen[d] = np.maximum(128, ((m + 127) // 128) * 128)

    colbase = np.zeros((2, NG, NSH), np.int64)   # column offset inside group
    SG = np.zeros((2, NG), np.int64)
    for d in range(2):
        for g in range(NG):
            cb = 0
            for t in range(NSH):
                colbase[d, g, t] = cb
                cb += seglen[d, g, t] // 128
            SG[d, g] = cb
    SGMAX = int(SG.max())

    # per-edge slot position within its group's stream
    for d in range(2):
        dd = dirs[d]
        segid = dd["core"] * (NG * NSH) + dd["seg"] % (NG * NSH)
        # rank within segment
        starts = np.searchsorted(segid, np.arange(n_cores * NG * NSH))
        rank = np.arange(segid.size) - starts[segid]
        dd["slot"] = colbase[d, dd["g"], dd["t"]] * 128 + rank
        dd["col"] = dd["slot"] >> 7

    # per-(d,g,q,t) column ranges (union over cores) and label offsets.
    # A block's edges form one contiguous run per source shard segment.
    c0 = np.zeros((2, NG, G, NSH), np.int64)
    nc_ = np.zeros((2, NG, G, NSH), np.int64)
    for d in range(2):
        dd = dirs[d]
        gqt = (dd["g"] * G + dd["q"]) * NSH + dd["t"]
        cmin = np.full(NG * G * NSH, 1 << 30, np.int64)
        cmax = np.full(NG * G * NSH, -1, np.int64)
        np.minimum.at(cmin, gqt, dd["col"])
        np.maximum.at(cmax, gqt, dd["col"])
        has = cmax >= 0
        c0[d][has.reshape(NG, G, NSH)] = cmin[has]
        nc_[d].reshape(-1)[has] = (cmax - cmin + 1)[has]
    # blocks with zero edges anywhere: give one inert column
    tot = nc_.sum(axis=3)
    for d in range(2):
        for g in range(NG):
            for q in range(G):
                if tot[d, g, q] == 0:
                    nc_[d, g, q, 0] = 1
    NCMAX = int(nc_.sum(axis=3).max())
    lblofs = np.zeros((2, NG, G), np.int64)       # label col offset per block
    tofsw = np.zeros((2, NG, G, NSH), np.int64)   # within-block per-shard ofs
    acc = 0
    for d in range(2):
        for g in range(NG):
            for q in range(G):
                lblofs[d, g, q] = acc
                w = 0
                for t in range(NSH):
                    tofsw[d, g, q, t] = w
                    w += nc_[d, g, q, t]
                acc += w
    LCOLS = int(acc)

    # group stream offsets (common to all cores)
    gofs = np.zeros((2, NG), np.int64)        # in slots
    itofs = np.zeros((2, NG, NSH), np.int64)  # idxT col offsets
    ikofs = np.zeros((2, NG), np.int64)       # idxK col offsets
    L = np.zeros(2, np.int64)
    ti, ki = 0, 0
    for d in range(2):
        o = 0
        for g in range(NG):
            gofs[d, g] = o
            for t in range(NSH):
                itofs[d, g, t] = ti
                ti += seglen[d, g, t] // 16
            ikofs[d, g] = ki
            ki += (SG[d, g] * 128) // 16
            o += SG[d, g] * 128
        L[d] = o
    TCOLS, KCOLS = int(ti), int(ki)

    # per-core streams
    idxT = np.full((n_cores, TCOLS * 16), RSH, np.int16)
    idxK = np.full((n_cores, KCOLS * 16), AKS, np.int16)
    lbl = np.full((n_cores, P, LCOLS), 999.0, np.float16)
    for d in range(2):
        dd = dirs[d]
        # stream position of each edge inside its core's direction stream
        spos = gofs[d, dd["g"]] + dd["slot"]
        # idxT: wrapped per segment; compute wrapped linear position:
        # within segment (g,t): j = slot - colbase*128 ... wrapped [16, len/16]
        j = dd["slot"] - colbase[d, dd["g"], dd["t"]] * 128
        wrap = (itofs[d, dd["g"], dd["t"]] + j // 16) * 16 + (j % 16)
        jk = dd["slot"]
        wrapk = (ikofs[d, dd["g"]] + jk // 16) * 16 + (jk % 16)
        pcol = (lblofs[d, dd["g"], dd["q"]] +
                tofsw[d, dd["g"], dd["q"], dd["t"]] +
                dd["col"] - c0[d, dd["g"], dd["q"], dd["t"]])
        for k in range(n_cores):
            m = dd["core"] == k
            idxT[k][wrap[m]] = dd["row16"][m]
            idxK[k][wrapk[m]] = dd["kloc"][m]
            lbl[k][dd["slot"][m] & 127, pcol[m]] = dd["klm"][m].astype(np.float16)
    # [16, cols] wrap -> replicate to 128 partitions
    idxT = np.ascontiguousarray(
        np.tile(idxT.reshape(n_cores, TCOLS, 16).transpose(0, 2, 1), (1, 8, 1)))
    idxK = np.ascontiguousarray(
        np.tile(idxK.reshape(n_cores, KCOLS, 16).transpose(0, 2, 1), (1, 8, 1)))

    meta.update(seglen=seglen, colbase=colbase, SG=SG, SGMAX=SGMAX, gofs=gofs,
                itofs=itofs, ikofs=ikofs, c0=c0, ncols=nc_, NCMAX=NCMAX,
                lblofs=lblofs, tofsw=tofsw, LCOLS=LCOLS, TCOLS=TCOLS,
                KCOLS=KCOLS)
    return meta, idxT, idxK, lbl


def _build(meta, n_cores, compile_only=True, build_stage=4):
    import concourse.bacc as bacc
    import concourse.mybir as mybir
    import concourse.tile as tile

    fp16 = mybir.dt.float16
    f32 = mybir.dt.float32
    i16 = mybir.dt.int16

    NBLK_C, NB, NPAD, RSH = meta["NBLK_C"], meta["NB"], meta["NPAD"], meta["RSH"]
    NG, G, AKS = meta["NG"], meta["G"], meta["AKS"]
    SGMAX, NCMAX, LCOLS = meta["SGMAX"], meta["NCMAX"], meta["LCOLS"]
    TCOLS, KCOLS = meta["TCOLS"], meta["KCOLS"]
    seglen, colbase, SG = meta["seglen"], meta["colbase"], meta["SG"]
    gofs, itofs, ikofs = meta["gofs"], meta["itofs"], meta["ikofs"]
    c0a, ncolsa, lblofs = meta["c0"], meta["ncols"], meta["lblofs"]
    RSHT = RSH + 1

    nc = bacc.Bacc(None, target_bir_lowering=False, debug=False,
                   num_swdge_queues=4)

    p_xT = nc.declare_dram_parameter("xT", [P, NPAD], fp16, isOutput=False)
    p_xo = nc.declare_dram_parameter("xo", [P, NBLK_C * P], fp16, isOutput=False)
    p_wcat = nc.declare_dram_parameter("wcat", [P, 256], fp16, isOutput=False)
    p_wdst = nc.declare_dram_parameter("wdst", [P, 16], fp16, isOutput=False)
    p_asrc = nc.declare_dram_parameter("asrcc", [P, 256], fp16, isOutput=False)
    p_iota = nc.declare_dram_parameter("iota", [P, NCMAX * P], fp16, isOutput=False)
    p_bias = nc.declare_dram_parameter("bias", [P, 128], f32, isOutput=False)
    p_zrow = nc.declare_dram_parameter("zrow", [1, 128], fp16, isOutput=False)
    p_nrow = nc.declare_dram_parameter("nrow", [1, 128], fp16, isOutput=False)
    p_idxT = nc.declare_dram_parameter("idxT", [P, TCOLS], i16, isOutput=False)
    p_idxK = nc.declare_dram_parameter("idxK", [P, KCOLS], i16, isOutput=False)
    p_lbl = nc.declare_dram_parameter("lbl", [P, LCOLS], fp16, isOutput=False)
    p_out = nc.declare_dram_parameter("out", [NBLK_C * P, 128], f32, isOutput=True)

    T = [nc.dram_tensor("T0", [NSH * RSHT, 128], fp16),
         nc.dram_tensor("T1", [NSH * RSHT, 128], fp16)]
    aK = nc.dram_tensor("aK", [NBLK_C * P + 1, 128], fp16)

    NBAT = 8

    with tile.TileContext(nc) as tc:
        with (
            tc.tile_pool(name="const", bufs=1) as cpool,
            tc.tile_pool(name="gh", bufs=3) as gpool,
            tc.tile_pool(name="agh", bufs=3) as apool,
            tc.tile_pool(name="tmp", bufs=1) as tpool,
            tc.tile_pool(name="msg", bufs=2) as mpool,
            tc.tile_pool(name="sm", bufs=2) as smpool,
            tc.tile_pool(name="oneh", bufs=3) as opool,
            tc.tile_pool(name="idx", bufs=2) as ipool,
            tc.tile_pool(name="epi", bufs=2) as dpool,
            tc.tile_pool(name="odp", bufs=2) as odpool,
            tc.tile_pool(name="pd", bufs=4, space="PSUM") as ppool,
        ):
            wcat_s = cpool.tile([P, 256], fp16)
            nc.sync.dma_start(out=wcat_s[:], in_=p_wcat[:])
            wdst_s = cpool.tile([P, 16], fp16)
            nc.sync.dma_start(out=wdst_s[:], in_=p_wdst[:])
            asrc_s = cpool.tile([P, 256], fp16)
            nc.sync.dma_start(out=asrc_s[:], in_=p_asrc[:])
            iota_s = cpool.tile([P, NCMAX * P], fp16)
            nc.sync.dma_start(out=iota_s[:], in_=p_iota[:])
            bias_s = cpool.tile([P, 128], f32)
            nc.sync.dma_start(out=bias_s[:], in_=p_bias[:])
            lbl_s = cpool.tile([P, LCOLS], fp16)
            nc.sync.dma_start(out=lbl_s[:], in_=p_lbl[:])

            # sentinel rows
            for d in range(2):
                for t in range(NSH):
                    nc.sync.dma_start(out=T[d][t * RSHT + RSH:t * RSHT + RSHT, :],
                                      in_=p_zrow[:])
            nc.sync.dma_start(out=aK[NBLK_C * P:NBLK_C * P + 1, :], in_=p_nrow[:])

            # ---------------- node phase ----------------
            with (
                tc.tile_pool(name="xc", bufs=2) as xpool,
                tc.tile_pool(name="stage", bufs=2) as spool,
                tc.tile_pool(name="nps", bufs=2, space="PSUM") as npsum,
            ):
                # aK (own blocks) first: unblocks edge-phase aK gathers early
                NB2 = 7
                for b0 in range(0, NBLK_C, NB2):
                    nb = min(NB2, NBLK_C - b0)
                    xo = xpool.tile([P, NB2 * P], fp16, tag="xo")
                    nc.sync.dma_start(out=xo[:, 0:nb * P],
                                      in_=p_xo[:, b0 * P:(b0 + nb) * P])
                    stga = spool.tile([P, NB2 * 16], fp16, tag="stga")
                    for j in range(nb):
                        psa = npsum.tile([P, 16], f32, tag="npsa")
                        nc.tensor.matmul(out=psa[:], lhsT=xo[:, j * P:(j + 1) * P],
                                         rhs=wdst_s[:], start=True, stop=True)
                        if j % 2 == 0:
                            nc.scalar.copy(out=stga[:, j * 16:(j + 1) * 16], in_=psa[:])
                        else:
                            nc.vector.tensor_copy(out=stga[:, j * 16:(j + 1) * 16],
                                                  in_=psa[:])
                    av = aK[b0 * P:(b0 + nb) * P, 0:16].rearrange(
                        "(j p) c -> p j c", p=P)
                    nc.sync.dma_start(
                        out=av, in_=stga[:].rearrange("p (j c) -> p j c", c=16)[:, 0:nb, :])

                # h tables: one pass per direction so T0 completes before
                # T1 work, letting d=0 edge gathers overlap the T1 pass
                for dp in range(2):
                    for g0 in range(0, NB, NBAT):
                        nb = min(NBAT, NB - g0)
                        xc = xpool.tile([P, NBAT * P], fp16, tag="xc")
                        nc.sync.dma_start(out=xc[:, 0:nb * P],
                                          in_=p_xT[:, g0 * P:(g0 + nb) * P])
                        stage = spool.tile([P, NBAT * 128], fp16, tag="stage")
                        for j in range(nb):
                            ps = npsum.tile([P, 128], f32, tag="nps")
                            nc.tensor.matmul(out=ps[:],
                                             lhsT=xc[:, j * P:(j + 1) * P],
                                             rhs=wcat_s[:, dp * 128:(dp + 1) * 128],
                                             start=True, stop=True)
                            dstg = stage[:, j * 128:(j + 1) * 128]
                            if j % 2 == 0:
                                nc.scalar.copy(out=dstg, in_=ps[:])
                            else:
                                nc.vector.tensor_copy(out=dstg, in_=ps[:])
                        st3 = stage[:].rearrange("p (j c) -> p j c", c=128)
                        for t in range(NSH):
                            dv = T[dp][t * RSHT:t * RSHT + RSH, :].rearrange(
                                "(q n) c -> q n c", n=NB)[:, g0:g0 + nb, :]
                            nc.sync.dma_start(
                                out=dv, in_=st3[32 * t:32 * (t + 1), 0:nb, :])

            # ---------------- edge phase ----------------
            if build_stage == 1:
                for b in range(NBLK_C):
                    ofin = dpool.tile([P, 128], f32, tag="ofin")
                    nc.vector.tensor_copy(out=ofin[:], in_=bias_s[:])
                    nc.sync.dma_start(out=p_out[b * P:(b + 1) * P, :], in_=ofin[:])
            for g in (range(NG) if build_stage >= 2 or build_stage in (20, 21, 22) else []):
                ods = {}
                for d in range(2):
                    Sg = int(SG[d, g])
                    gTcols = Sg * 8
                    idxt = ipool.tile([P, SGMAX * 8], i16, tag="idxt")
                    nc.scalar.dma_start(
                        out=idxt[:, 0:gTcols],
                        in_=p_idxT[:, int(itofs[d, g, 0]):int(itofs[d, g, 0]) + gTcols])
                    idxk = ipool.tile([P, SGMAX * 8], i16, tag="idxk")
                    nc.scalar.dma_start(
                        out=idxk[:, 0:gTcols],
                        in_=p_idxK[:, int(ikofs[d, g]):int(ikofs[d, g]) + gTcols])

                    gh = gpool.tile([P, SGMAX * P], fp16, tag="gh")
                    gh3 = gh[:].rearrange("p (s c) -> p s c", c=P)
                    if build_stage == 21:
                        nc.vector.tensor_scalar(
                            out=gh[:], in0=iota_s[:, 0:1].to_broadcast([P, SGMAX * P]),
                            scalar1=0.0, scalar2=None, op0=mybir.AluOpType.mult)
                    for t in (range(NSH) if build_stage != 21 else []):
                        sl = int(seglen[d, g, t])
                        cb = int(colbase[d, g, t])
                        io = int(itofs[d, g, t] - itofs[d, g, 0])
                        base = 0 if build_stage == 22 else t * RSHT
                        nc.gpsimd.dma_gather(
                            out_ap=gh3[:, cb:cb + sl // 128, :],
                            in_ap=T[d][base:base + RSHT, :],
                            idxs_ap=idxt[:, io:io + sl // 16],
                            num_idxs=sl, num_idxs_reg=sl, elem_size=P,
                            single_packet=False, queue_num=t % 4)
                    agh = apool.tile([P, SGMAX * P], fp16, tag="agh")
                    agh3 = agh[:].rearrange("p (s c) -> p s c", c=P)
                    if build_stage != 20:
                        # split across all 4 SWDGE queues to balance per-queue
                        # descriptor load with the four T-gathers
                        ck = (Sg + 3) // 4
                        cb2 = 0
                        for ci in range(4):
                            cw = min(ck, Sg - cb2)
                            if cw <= 0:
                                break
                            nc.gpsimd.dma_gather(
                                out_ap=agh3[:, cb2:cb2 + cw, :],
                                in_ap=aK[0:NBLK_C * P + 1, :],
                                idxs_ap=idxk[:, cb2 * 8:(cb2 + cw) * 8],
                                num_idxs=cw * 128, num_idxs_reg=cw * 128,
                                elem_size=P, single_packet=False,
                                queue_num=(d + ci) % 4)
                            cb2 += cw
                    else:
                        nc.vector.tensor_scalar(out=agh[:], in0=gh[:],
                                                scalar1=0.0, scalar2=None,
                                                op0=mybir.AluOpType.mult)

                    if build_stage in (2, 20, 21, 22):
                        od2 = odpool.tile([P, 128], f32, tag=f"od{d}_0s2")
                        nc.vector.tensor_copy(out=od2[:], in_=gh[:, 0:128])
                        od2b = odpool.tile([P, 128], f32, tag=f"od{d}_1s2")
                        nc.vector.tensor_copy(out=od2b[:], in_=agh[:, 0:128])
                        for q in range(G):
                            ods[(d, q)] = od2 if q % 2 == 0 else od2b
                        continue
                    # alpha_src = <h, a_src_d> per head
                    tmp = tpool.tile([P, SGMAX * P], fp16, tag="tmp")
                    nc.vector.tensor_tensor(
                        out=tmp[:, 0:Sg * P].rearrange("p (s h c) -> p s h c",
                                                       h=HEADS, c=C),
                        in0=gh[:, 0:Sg * P].rearrange("p (s h c) -> p s h c",
                                                      h=HEADS, c=C),
                        in1=asrc_s[:, d * 128:(d + 1) * 128].rearrange(
                            "p (o h c) -> p o h c", o=1, h=HEADS, c=C
                        ).to_broadcast([P, Sg, HEADS, C]),
                        op=mybir.AluOpType.mult)
                    asr = smpool.tile([P, SGMAX * 8], f32, tag="asr")
                    nc.vector.tensor_reduce(
                        out=asr[:, 0:Sg * 8],
                        in_=tmp[:, 0:Sg * P].rearrange("p (sh c) -> p sh c", c=C),
                        axis=mybir.AxisListType.X, op=mybir.AluOpType.add)

                    aex = smpool.tile([P, SGMAX * 8], fp16, tag="aex")
                    nc.vector.tensor_tensor(
                        out=aex[:, 0:Sg * 8].rearrange("p (s h) -> p s h", h=8),
                        in0=asr[:, 0:Sg * 8].rearrange("p (s h) -> p s h", h=8),
                        in1=agh3[:, 0:Sg, d * 8:(d + 1) * 8],
                        op=mybir.AluOpType.add)
                    lrl0 = smpool.tile([P, SGMAX * 8], fp16, tag="lrl0")
                    nc.vector.tensor_scalar(out=lrl0[:, 0:Sg * 8],
                                            in0=aex[:, 0:Sg * 8],
                                            scalar1=NEG_SLOPE, scalar2=None,
                                            op0=mybir.AluOpType.mult)
                    lrl = smpool.tile([P, SGMAX * 8], fp16, tag="lrl")
                    nc.vector.tensor_tensor(out=lrl[:, 0:Sg * 8],
                                            in0=aex[:, 0:Sg * 8],
                                            in1=lrl0[:, 0:Sg * 8],
                                            op=mybir.AluOpType.max)
                    ex = smpool.tile([P, SGMAX * 8], fp16, tag="ex")
                    nc.scalar.activation(out=ex[:, 0:Sg * 8], in_=lrl[:, 0:Sg * 8],
                                         func=mybir.ActivationFunctionType.Exp)

                    # msg slots: [ex*h (128) | ex (8)]
                    msg = mpool.tile([P, SGMAX * 136], fp16, tag="msg")
                    msg3 = msg[:].rearrange("p (s c) -> p s c", c=136)
                    nc.vector.tensor_tensor(
                        out=msg3[:, 0:Sg, 0:128].rearrange("p s (h c) -> p s h c",
                                                           c=C),
                        in0=gh3[:, 0:Sg, :].rearrange("p s (h c) -> p s h c", c=C),
                        in1=ex[:, 0:Sg * 8].rearrange("p (s h o) -> p s h o",
                                                      h=8, o=1
                                                      ).to_broadcast([P, Sg, 8, C]),
                        op=mybir.AluOpType.mult)
                    nc.vector.tensor_copy(
                        out=msg3[:, 0:Sg, 128:136],
                        in_=ex[:, 0:Sg * 8].rearrange("p (s h) -> p s h", h=8))

                    if build_stage == 3:
                        for q in range(G):
                            od3 = odpool.tile([P, 128], f32, tag=f"od{d}_{q}")
                            nc.vector.tensor_copy(out=od3[:], in_=msg[:, 0:128])
                            ods[(d, q)] = od3
                        continue
                    for q in range(G):
                        ncq = int(ncolsa[d, g, q].sum())
                        lo = int(lblofs[d, g, q])
                        oneh = opool.tile([P, NCMAX * P], fp16, tag="oneh")
                        nc.vector.tensor_tensor(
                            out=oneh[:, 0:ncq * P].rearrange("p (n e) -> p n e",
                                                             e=P),
                            in0=lbl_s[:, lo:lo + ncq].rearrange(
                                "p (n o) -> p n o", o=1).to_broadcast([P, ncq, P]),
                            in1=iota_s[:, 0:ncq * P].rearrange("p (n e) -> p n e",
                                                               e=P),
                            op=mybir.AluOpType.is_equal)
                        pd = ppool.tile([P, 136], f32, tag="pd")
                        i = 0
                        for t in range(NSH):
                            for w in range(int(ncolsa[d, g, q, t])):
                                cc = int(c0a[d, g, q, t]) + w
                                nc.tensor.matmul(
                                    out=pd[:],
                                    lhsT=oneh[:, i * P:(i + 1) * P],
                                    rhs=msg[:, cc * 136:(cc + 1) * 136],
                                    start=(i == 0), stop=(i == ncq - 1))
                                i += 1
                        den = dpool.tile([P, 8], f32, tag="den")
                        nc.vector.tensor_scalar(out=den[:], in0=pd[:, 128:136],
                                                scalar1=1e-30, scalar2=None,
                                                op0=mybir.AluOpType.add)
                        rec = dpool.tile([P, 8], f32, tag="rec")
                        nc.vector.reciprocal(out=rec[:], in_=den[:])
                        od = odpool.tile([P, 128], f32, tag=f"od{d}_{q}")
                        nc.vector.tensor_tensor(
                            out=od[:].rearrange("p (h c) -> p h c", c=C),
                            in0=pd[:, 0:128].rearrange("p (h c) -> p h c", c=C),
                            in1=rec[:].rearrange("p (h o) -> p h o", o=1
                                                 ).to_broadcast([P, 8, C]),
                            op=mybir.AluOpType.mult)
                        ods[(d, q)] = od

                for q in range(G):
                    osum = dpool.tile([P, 128], f32, tag="osum")
                    nc.vector.tensor_tensor(out=osum[:], in0=ods[(0, q)][:],
                                            in1=ods[(1, q)][:],
                                            op=mybir.AluOpType.add)
                    ofin = dpool.tile([P, 128], f32, tag="ofin")
                    nc.vector.tensor_tensor(out=ofin[:], in0=osum[:], in1=bias_s[:],
                                            op=mybir.AluOpType.add)
                    b = g * G + q
                    nc.sync.dma_start(out=p_out[b * P:(b + 1) * P, :], in_=ofin[:])

    nc.compile()
    return nc


def _host_inputs(meta, x, W_in, a_src_in, a_dst_in, b_in, W_out, a_src_out,
                 a_dst_out, b_out, idxT, idxK, lbl, n_nodes, n_cores):
    NB, NPAD, NBLK_C, NCMAX = meta["NB"], meta["NPAD"], meta["NBLK_C"], meta["NCMAX"]
    Vdst_in = np.stack([W_in[:, h * C:(h + 1) * C] @ a_dst_in[h]
                        for h in range(HEADS)], 1)   # [D_IN, HEADS]
    Vdst_out = np.stack([W_out[:, h * C:(h + 1) * C] @ a_dst_out[h]
                         for h in range(HEADS)], 1)
    wcat = np.concatenate([W_in, W_out], axis=1).astype(np.float16)
    wdst = np.concatenate([Vdst_in, Vdst_out], axis=1).astype(np.float16)
    asrcc = np.tile(np.concatenate([a_src_in.reshape(-1), a_src_out.reshape(-1)]
                                   ).astype(np.float16)[None, :], (P, 1))
    xT = np.zeros((D_IN, NPAD), np.float16)
    xT[:, :n_nodes] = x.T.astype(np.float16)
    iota = np.tile(np.arange(P, dtype=np.float16), (P, NCMAX)).reshape(P, NCMAX * P)
    bias = np.tile((b_in + b_out).astype(np.float32)[None, :], (P, 1))
    zrow = np.zeros((1, 128), np.float16)
    nrow = np.full((1, 128), -30000.0, np.float16)

    shared = dict(xT=xT, wcat=wcat, wdst=wdst, asrcc=asrcc, iota=iota,
                  bias=bias, zrow=zrow, nrow=nrow)
    in_maps = []
    for k in range(n_cores):
        xo = xT[:, k * NBLK_C * P:(k + 1) * NBLK_C * P]
        in_maps.append(dict(shared, xo=np.ascontiguousarray(xo),
                            idxT=idxT[k], idxK=idxK[k], lbl=lbl[k]))
    return in_maps


def kernel(x, ei, W_in, a_src_in, a_dst_in, b_in, W_out, a_src_out, a_dst_out,
           b_out, n_cores=8, G=7):
    from concourse.bass_utils import run_bass_kernel_spmd

    x = np.asarray(x, np.float32)
    ei = np.asarray(ei, np.int32)
    n_nodes = x.shape[0]
    meta, idxT, idxK, lbl = _prep(ei, n_nodes, n_cores, G)
    nc = _build(meta, n_cores)
    in_maps = _host_inputs(meta, x,
                           np.asarray(W_in, np.float32),
                           np.asarray(a_src_in, np.float32),
                           np.asarray(a_dst_in, np.float32),
                           np.asarray(b_in, np.float32),
                           np.asarray(W_out, np.float32),
                           np.asarray(a_src_out, np.float32),
                           np.asarray(a_dst_out, np.float32),
                           np.asarray(b_out, np.float32),
                           idxT, idxK, lbl, n_nodes, n_cores)
    _LAST.update(nc=nc, in_maps=in_maps, n_cores=n_cores, meta=meta)
    res = run_bass_kernel_spmd(nc, in_maps, list(range(n_cores)))
    full = np.concatenate([res.results[k]["out"] for k in range(n_cores)], axis=0)
    return full[:n_nodes].astype(np.float32)
